# revision 48
# baseline (speedup 1.0000x reference)
"""Distributed Bass kernel: 3D windowed attention with decomposed rel-pos bias.

Sharding: 8 cores = 4 batches x 2 head-groups (6 heads each).
Per-core layout is fully transposed ([channel, token]); the rel-pos bias is
folded into the scores matmul as 36 extra contraction channels (one-hot
k-position rows in the stationary operand, F = q.R tables in the moving
operand).  Softmax runs max-free; the denominator comes free as a ones-row
appended to V in the AV matmul.  All matmuls run in bf16 with fp32 PSUM.

v2 restructure (from trace analysis of the 448us baseline):
- scores PSUM is a [128, 784] 2-bank tile; ONE 784-wide Exp ACT per
  (head, q-half, k-tile) instead of four 512/32-wide ones (scalar engine
  fixed cost is ~352cyc/instr; this cuts scalar time ~40%).
- softmax 1/denom via DVE reciprocal_approx_fast instead of scalar
  Ln+Exp: removes 12 ACT_TABLE_LOAD bounces (~32us of scalar).
- V projection bias folded into the host-side output bias (rows of a
  normalized softmax sum to 1), removing per-head Identity ACTs.
- next pair's QKV/F matmuls are dribbled into the attention k-loop every
  few k-tiles so the PE never idles (HAM clock-gate stays warm).
- aug assembly copies moved to DMA (q rows, one-hot rows) or merged
  (both heads per F copy, V copy via strided dest) to unload the DVE.
"""

import os
import sys

import numpy as np

sys.path.insert(0, "/opt/trn_rl_repo")

B, D, H, W, C = 4, 8, 14, 14, 768
NH, HD = 12, 64
N = D * H * W  # 1568
HPC = 6  # heads per core
SCALE = HD ** -0.5
NKC = C // 128  # 6 k-chunks of input channels
NKT = (N + 127) // 128  # 13 token tiles (12x128 + 32)
FCH = [(0, 512), (512, 512), (1024, 512), (1536, 32)]  # free-dim chunks of N
QHW = 784  # q columns per attention half
HCH = [(0, 512), (512, 272)]  # free-dim chunks of a 784 half
KAUG = 100  # 64 qk channels + 8 + 14 + 14 bias channels

_CACHED = {}


def _build_nc():
    import concourse.bass as bass  # noqa: F401
    import concourse.mybir as mybir
    import concourse.tile as tile
    from concourse import bacc

    f32 = mybir.dt.float32
    bf16 = mybir.dt.bfloat16
    AF = mybir.ActivationFunctionType

    nc = bacc.Bacc(None, target_bir_lowering=False)

    # --- DRAM parameters (per-core shards; host pre-transposes/reorders) ---
    xT_d = nc.declare_dram_parameter("xT", [NKC, 128, N], bf16, isOutput=False)
    wqkv_d = nc.declare_dram_parameter("wqkv", [NKC, 128, 1152], bf16, isOutput=False)
    wproj_d = nc.declare_dram_parameter("wproj", [3, 128, 768], bf16, isOutput=False)
    oneh_d = nc.declare_dram_parameter("oneh", [36, N], bf16, isOutput=False)
    rdT_d = nc.declare_dram_parameter("rdT", [128, D * 40], bf16, isOutput=False)
    rhT_d = nc.declare_dram_parameter("rhT", [128, H * 46], bf16, isOutput=False)
    rwT_d = nc.declare_dram_parameter("rwT", [128, W * 46], bf16, isOutput=False)
    bqk_d = nc.declare_dram_parameter("bqk", [128, 6], f32, isOutput=False)
    out_d = nc.declare_dram_parameter("out", [NKC, 128, N], bf16, isOutput=True)

    with tile.TileContext(nc) as tc:
        with (
            tc.tile_pool(name="const", bufs=1) as cpool,
            tc.tile_pool(name="work", bufs=2) as wpool,
            tc.tile_pool(name="qkp", bufs=2, space="PSUM") as qkp,
            tc.tile_pool(name="avp", bufs=1, space="PSUM") as avp,
            tc.tile_pool(name="mmp", bufs=2, space="PSUM") as mmp,
        ):
            # ---- load constants ----
            xT = cpool.tile([128, NKC * N], bf16)
            wqkv = cpool.tile([128, NKC * 1152], bf16)
            wproj = cpool.tile([128, 3 * 768], bf16)
            oneh = cpool.tile([36, N], bf16)
            rdT = cpool.tile([128, D * 40], bf16)
            rhT = cpool.tile([128, H * 46], bf16)
            rwT = cpool.tile([128, W * 46], bf16)
            bqk = cpool.tile([128, 6], f32)
            # xT/wqkv land first (first matmuls need all of both); wproj last
            for kc in range(NKC):
                nc.sync.dma_start(xT[:, kc * N:(kc + 1) * N], xT_d[kc])
                nc.sync.dma_start(wqkv[:, kc * 1152:(kc + 1) * 1152], wqkv_d[kc])
            nc.sync.dma_start(bqk[:], bqk_d[:])
            nc.sync.dma_start(rdT[:], rdT_d[:])
            nc.sync.dma_start(rhT[:], rhT_d[:])
            nc.sync.dma_start(rwT[:], rwT_d[:])
            nc.sync.dma_start(oneh[:], oneh_d[:])
            for t3 in range(3):
                nc.sync.dma_start(wproj[:, t3 * 768:(t3 + 1) * 768], wproj_d[t3])

            # ---- persistent SBUF state ----
            # V in natural [token, channel] layout, ones column per head
            vnat = cpool.tile([128, NKT, HPC * 65], bf16)
            av_all = [
                cpool.tile([128, N], bf16, name=f"av_all{i}", tag=f"av{i}")
                for i in range(3)
            ]

            # ---- deferred-work queue (dribbled into attention k-loops) ----
            pending = []

            def dribble():
                if pending:
                    pending.pop(0)()

            def emit_v(kt):
                kp = min(128, N - kt * 128)
                pv = mmp.tile([128, 512], f32, name="pv", tag="mm")
                for kc in range(NKC):
                    nc.tensor.matmul(
                        pv[0:kp, 0:384],
                        xT[:, kc * N + kt * 128: kc * N + kt * 128 + kp],
                        wqkv[:, kc * 1152 + 768: kc * 1152 + 1152],
                        start=(kc == 0), stop=(kc == NKC - 1),
                    )
                dst = vnat[0:kp, kt, :].rearrange("p (h c) -> p h c", h=HPC)
                src = pv[0:kp, 0:384].rearrange("p (h c) -> p h c", h=HPC)
                nc.vector.tensor_copy(dst[:, :, 0:64], src)

            # ---- QKV for one head pair (queued as closures) ----
            def emit_qkv(p, eager):
                augs = []
                for x in range(2):
                    q_t = wpool.tile([KAUG, N], bf16, name=f"qaug{x}", tag=f"qaug{x}")
                    k_t = wpool.tile([128, N], bf16, name=f"kaug{x}", tag=f"kaug{x}")
                    augs.append((q_t, k_t))
                qpair = wpool.tile([128, N], bf16, name="qpair", tag="qpair")

                def chunk(qk, f0, fl):
                    def run():
                        col0 = qk * 384 + p * 128
                        bcol = qk * 3 + p
                        ps = mmp.tile([128, 512], f32, name="ps", tag="mm")
                        for kc in range(NKC):
                            nc.tensor.matmul(
                                ps[:, 0:fl],
                                wqkv[:, kc * 1152 + col0: kc * 1152 + col0 + 128],
                                xT[:, kc * N + f0: kc * N + f0 + fl],
                                start=(kc == 0), stop=(kc == NKC - 1),
                            )
                        # pair 0's bias-adds ride the then-idle scalar engine
                        # so the DVE can assemble the aug tiles sooner
                        if qk == 0:
                            if eager:
                                nc.scalar.add(
                                    qpair[:, f0:f0 + fl], ps[:, 0:fl],
                                    bqk[:, bcol:bcol + 1],
                                )
                            else:
                                nc.vector.tensor_scalar_add(
                                    qpair[:, f0:f0 + fl], ps[:, 0:fl],
                                    bqk[:, bcol:bcol + 1],
                                )
                        else:
                            for x in range(2):
                                if eager:
                                    nc.scalar.add(
                                        augs[x][1][0:64, f0:f0 + fl],
                                        ps[x * 64:(x + 1) * 64, 0:fl],
                                        bqk[x * 64:(x + 1) * 64, bcol:bcol + 1],
                                    )
                                else:
                                    nc.vector.tensor_scalar_add(
                                        augs[x][1][0:64, f0:f0 + fl],
                                        ps[x * 64:(x + 1) * 64, 0:fl],
                                        bqk[x * 64:(x + 1) * 64, bcol:bcol + 1],
                                    )
                    return run

                def assemble():
                    for x in range(2):
                        nc.sync.dma_start(
                            augs[x][0][0:64, :], qpair[x * 64:(x + 1) * 64, :]
                        )
                        nc.sync.dma_start(augs[x][1][64:KAUG, :], oneh[:])

                jobs = [chunk(qk, f0, fl) for qk in range(2) for (f0, fl) in FCH]
                jobs.append(assemble)
                if eager:
                    for j in jobs:
                        j()
                else:
                    pending.extend(jobs)
                return augs, qpair

            # ---- F = q.R rel-pos tables for one pair (queued as closures) ----
            def emit_F(p, augs, qpair, eager):
                qpv = qpair.rearrange("p (d h w) -> p d h w", d=D, h=H, w=W)

                fsD = wpool.tile([40, N], bf16, name="fsD", tag="fsD")
                fsH = wpool.tile([46, N], bf16, name="fsH", tag="fsH")
                fsW = wpool.tile([46, N], bf16, name="fsW", tag="fsW")
                fvH = fsH.rearrange("p (d h w) -> p d h w", d=D, h=H, w=W)
                fvW = fsW.rearrange("p (d h w) -> p d h w", d=D, h=H, w=W)

                def rel_d(qds):
                    def run():
                        for qd in qds:
                            pf = mmp.tile([128, 512], f32, name="pf", tag="mm")
                            nc.tensor.matmul(
                                pf[0:40, 0:H * W],
                                rdT[:, qd * 40:(qd + 1) * 40],
                                qpair[:, qd * H * W:(qd + 1) * H * W],
                            )
                            nc.vector.tensor_copy(
                                fsD[:, qd * H * W:(qd + 1) * H * W], pf[0:40, 0:H * W]
                            )
                    return run

                def rel_h(qhs):
                    def run():
                        for qh in qhs:
                            pf = mmp.tile([128, 512], f32, name="pf", tag="mm")
                            nc.tensor.matmul(
                                pf[0:46, 0:D * W], rhT[:, qh * 46:(qh + 1) * 46],
                                qpv[:, :, qh, :]
                            )
                            nc.vector.tensor_copy(fvH[:, :, qh, :], pf[0:46, 0:D * W])
                    return run

                def rel_w(qws):
                    def run():
                        for qw in qws:
                            pf = mmp.tile([128, 512], f32, name="pf", tag="mm")
                            nc.tensor.matmul(
                                pf[0:46, 0:D * H], rwT[:, qw * 46:(qw + 1) * 46],
                                qpv[:, :, :, qw]
                            )
                            nc.vector.tensor_copy(fvW[:, :, :, qw], pf[0:46, 0:D * H])
                    return run

                def ship():
                    for x in range(2):
                        nc.sync.dma_start(augs[x][0][64:72, :], fsD[32 * x:32 * x + 8, :])
                        nc.sync.dma_start(augs[x][0][72:86, :], fsH[32 * x:32 * x + 14, :])
                        nc.sync.dma_start(augs[x][0][86:100, :], fsW[32 * x:32 * x + 14, :])

                jobs = [rel_d(range(0, 4)), rel_d(range(4, 8)),
                        rel_h(range(0, 5)), rel_h(range(5, 10)), rel_h(range(10, 14)),
                        rel_w(range(0, 5)), rel_w(range(5, 10)), rel_w(range(10, 14)),
                        ship]
                if eager:
                    for j in jobs:
                        j()
                else:
                    pending.extend(jobs)

            # ---- attention for one head over one 784-wide q half ----
            def emit_att(p, x, qh, avst, denb, jit_v=False):
                h6 = 2 * p + x
                q_t, k_t = augs_cur[x]
                pav = avp.tile([65, QHW], f32, name="pav", tag="av")
                for kt in range(NKT):
                    kp = min(128, N - kt * 128)
                    if jit_v:
                        emit_v(kt)
                    elif kt % 2 == 1:
                        dribble()
                    qk = qkp.tile([128, QHW], f32, name="qk", tag="qk")
                    for (f0, fl) in HCH:
                        nc.tensor.matmul(
                            qk[0:kp, f0:f0 + fl],
                            k_t[0:KAUG, kt * 128: kt * 128 + kp],
                            q_t[0:KAUG, qh * QHW + f0: qh * QHW + f0 + fl],
                        )
                    et = wpool.tile([128, QHW], bf16, name="et", tag="et", bufs=4)
                    nc.scalar.activation(et[0:kp, :], qk[0:kp, 0:QHW], AF.Exp)
                    for (f0, fl) in HCH:
                        nc.tensor.matmul(
                            pav[:, f0:f0 + fl],
                            vnat[0:kp, kt, h6 * 65:(h6 + 1) * 65],
                            et[0:kp, f0:f0 + fl],
                            start=(kt == 0), stop=(kt == NKT - 1),
                        )
                # drain av rows + denominator row; recip needs its input at
                # partition 0, so the denominator row gets its own copy
                nc.vector.tensor_copy(
                    avst[0:64, qh * QHW:(qh + 1) * QHW], pav[0:64, :]
                )
                nc.vector.tensor_copy(
                    denb[0:1, qh * QHW:(qh + 1) * QHW], pav[64:65, :]
                )
                dribble()

            # ---- normalize one q-half of one head into av_all ----
            def emit_norm(p, x, qh, avst, denb):
                sl = slice(qh * QHW, (qh + 1) * QHW)
                rcp = wpool.tile([1, QHW], f32, name="rcp", tag="rcp")
                rbc = wpool.tile([64, QHW], f32, name="rbc", tag="rbc")
                nc.vector.reciprocal_approx_fast(rcp[0:1, :], denb[0:1, sl])
                nc.gpsimd.partition_broadcast(rbc[0:64, :], rcp[0:1, :])
                nc.vector.tensor_mul(
                    av_all[p][x * 64:(x + 1) * 64, sl], avst[0:64, sl], rbc[0:64, :]
                )

            # ---- projection helpers ----
            ots = [
                cpool.tile([128, N], bf16, name=f"ot{mo}", tag=f"out{mo}")
                for mo in range(NKC)
            ]

            def proj_chunk(mo, f0, fl, alt=0):
                def run():
                    # alternate PSUM pool and copy engine for a deeper
                    # tail pipeline (avp's banks are free by proj time)
                    pool = mmp if alt % 2 == 0 else avp
                    tag = "mm" if alt % 2 == 0 else "av"
                    ps = pool.tile([128, 512], f32, name="ps", tag=tag)
                    for t3 in range(3):
                        nc.tensor.matmul(
                            ps[:, 0:fl],
                            wproj[:, t3 * 768 + mo * 128: t3 * 768 + mo * 128 + 128],
                            av_all[t3][:, f0:f0 + fl],
                            start=(t3 == 0), stop=(t3 == 2),
                        )
                    if alt % 2 == 0:
                        nc.vector.tensor_copy(ots[mo][:, f0:f0 + fl], ps[:, 0:fl])
                    else:
                        nc.scalar.copy(ots[mo][:, f0:f0 + fl], ps[:, 0:fl])
                return run

            # ---- pipeline driver ----
            augs_cur, qpair_cur = emit_qkv(0, eager=True)
            emit_F(0, augs_cur, qpair_cur, eager=True)
            nc.vector.memset(vnat[:], 1.0)
            for p in range(3):
                avsts = [
                    wpool.tile([64, N], f32, name=f"avst{x}", tag=f"avst{x}", bufs=1)
                    for x in range(2)
                ]
                denbs = [
                    wpool.tile([1, N], f32, name=f"denb{x}", tag=f"denb{x}", bufs=1)
                    for x in range(2)
                ]
                if p < 2:
                    augs_nxt, qpair_nxt = emit_qkv(p + 1, eager=False)
                    emit_F(p + 1, augs_nxt, qpair_nxt, eager=False)
                for x in range(2):
                    for qh in range(2):
                        if p == 2 and x == 1 and qh == 1:
                            # proj cols 0:784 only need the qh=0 halves.
                            # alt=0 only: avp's slot is still held by pav here.
                            pending.extend(
                                proj_chunk(mo, f0, fl, alt=0)
                                for mo in range(NKC)
                                for (f0, fl) in ((0, 512), (512, 272))
                            )
                        emit_att(p, x, qh, avsts[x], denbs[x],
                                 jit_v=(p == 0 and x == 0 and qh == 0))
                        emit_norm(p, x, qh, avsts[x], denbs[x])
                if p < 2:
                    while pending:
                        dribble()
                    augs_cur, qpair_cur = augs_nxt, qpair_nxt

            # ---- rest of the partial projection: outT[768, N] ----
            while pending:
                dribble()
            alt = 0
            for mo in range(NKC):
                for (f0, fl) in ((784, 240), (1024, 512), (1536, 32)):
                    proj_chunk(mo, f0, fl, alt=alt)()
                    alt += 1
                nc.sync.dma_start(out_d[mo], ots[mo][:])

    nc.compile()
    return nc


def _prep_inputs(x, qkv_w, qkv_b, proj_w, proj_b, rel_pos_d, rel_pos_h, rel_pos_w):
    """Host-side shard prep: returns in_maps list for 8 cores."""
    import ml_dtypes
    bf = ml_dtypes.bfloat16
    x = np.ascontiguousarray(x, np.float32)
    qkv_w = np.asarray(qkv_w, np.float32)
    qkv_b = np.asarray(qkv_b, np.float32)
    proj_w = np.asarray(proj_w, np.float32)

    # one-hot k-position rows [36, N]
    j = np.arange(N)
    kd, kh, kw = j // (H * W), (j // W) % H, j % W
    oneh = np.zeros((36, N), np.float32)
    oneh[kd, j] = 1.0
    oneh[8 + kh, j] = 1.0
    oneh[22 + kw, j] = 1.0
    oneh = oneh.astype(bf)

    # rel tables, transposed and un-scaled (q is pre-scaled by SCALE).
    # Block-diagonal over the head pair: head A channels in rows 0:64 feed
    # psum rows 0:n, head B channels in rows 64:128 feed psum rows 32:32+n.
    def rtab(table, n, span):
        t = np.asarray(table, np.float32) / SCALE  # [2n-1, 64]
        qq, kk = np.meshgrid(np.arange(n), np.arange(n), indexing="ij")
        base = t[(qq - kk + n - 1).reshape(-1)].T.reshape(64, n, n)  # [c, q, k]
        out = np.zeros((128, n, span), np.float32)
        out[0:64, :, 0:n] = base
        out[64:128, :, 32:32 + n] = base
        return np.ascontiguousarray(out.reshape(128, n * span)).astype(bf)

    rdT = rtab(rel_pos_d, D, 40)
    rhT = rtab(rel_pos_h, H, 46)
    rwT = rtab(rel_pos_w, W, 46)

    in_maps = []
    for core in range(8):
        b, g = divmod(core, 2)
        heads = list(range(g * HPC, (g + 1) * HPC))
        # W columns: [q(6x64) | k(6x64) | v(6x64)] for this head group; q scaled
        cols_q = [0 * C + h * HD + c for h in heads for c in range(HD)]
        cols_k = [1 * C + h * HD + c for h in heads for c in range(HD)]
        cols_v = [2 * C + h * HD + c for h in heads for c in range(HD)]
        wq = qkv_w[:, cols_q] * SCALE
        wk = qkv_w[:, cols_k]
        wv = qkv_w[:, cols_v]
        wc = np.concatenate([wq, wk, wv], axis=1)  # [768, 1152]
        wqkv = np.ascontiguousarray(wc.reshape(NKC, 128, 1152)).astype(bf)

        bq = qkv_b[cols_q] * SCALE
        bk = qkv_b[cols_k]
        bqk = np.zeros((128, 6), np.float32)
        for p in range(3):
            bqk[:, p] = bq[p * 128:(p + 1) * 128]
            bqk[:, 3 + p] = bk[p * 128:(p + 1) * 128]

        rows = [h * HD + c for h in heads for c in range(HD)]
        wp = np.ascontiguousarray(proj_w[rows].reshape(3, 128, 768)).astype(bf)

        xT = np.ascontiguousarray(
            x[b].reshape(N, C).T.reshape(NKC, 128, N)
        ).astype(bf)
        in_maps.append({
            "xT": xT, "wqkv": wqkv, "wproj": wp, "oneh": oneh,
            "rdT": rdT, "rhT": rhT, "rwT": rwT, "bqk": bqk,
        })
    return in_maps


def _install_ntff_hook_shim():
    """The image's antenv package lacks axon_hooks; recreate it so
    run_bass_kernel_spmd(trace=True) can reach the libaxon NTFF profiler."""
    import types

    if "antenv.axon_hooks" in sys.modules:
        return
    import antenv
    mod = types.ModuleType("antenv.axon_hooks")
    _hook = [None]
    mod.set_axon_ntff_profile_hook = lambda h: _hook.__setitem__(0, h)
    mod.get_axon_ntff_profile_hook = lambda: _hook[0]
    antenv.axon_hooks = mod
    sys.modules["antenv.axon_hooks"] = mod
    try:
        from trn_agent_boot.trn_boot import _ntff_profile_via_ctypes

        mod.set_axon_ntff_profile_hook(
            _ntff_profile_via_ctypes("/opt/axon/libaxon_pjrt.so")
        )
    except Exception as e:  # degrade to no tracing
        print(f"ntff hook shim failed: {e}", file=sys.stderr)


def _patch_ldw_opt():
    """Enable walrus's LDWEIGHTS dedup (consecutive matmuls reusing the same
    stationary operand skip the reload). bass_utils hardcodes it off.
    Disabled: walrus codegen fails with it on for this BIR."""
    if not bool(int(os.environ.get("KERNEL_LDW_OPT", "0"))):
        return
    import concourse.bass_utils as bu

    if getattr(bu, "_ldw_patched", False):
        return
    orig = bu.run_command

    def run_command(cmd, *a, **kw):
        if isinstance(cmd, list):
            cmd = [
                "--enable-ldw-opt=true" if c == "--enable-ldw-opt=false" else c
                for c in cmd
            ]
        return orig(cmd, *a, **kw)

    bu.run_command = run_command
    bu._ldw_patched = True


LAST_EXEC_NS = None


def kernel(x, qkv_w, qkv_b, proj_w, proj_b, rel_pos_d, rel_pos_h, rel_pos_w):
    global LAST_EXEC_NS
    if "nc" not in _CACHED:
        _CACHED["nc"] = _build_nc()
    nc = _CACHED["nc"]
    in_maps = _prep_inputs(
        x, qkv_w, qkv_b, proj_w, proj_b, rel_pos_d, rel_pos_h, rel_pos_w
    )
    _patch_ldw_opt()
    from concourse.bass_utils import run_bass_kernel_spmd

    trace = bool(int(os.environ.get("KERNEL_TRACE", "0")))
    if trace:
        _install_ntff_hook_shim()
    res = run_bass_kernel_spmd(nc, in_maps, core_ids=list(range(8)), trace=trace)
    LAST_EXEC_NS = res.exec_time_ns
    # V-projection bias: softmax rows sum to 1, so attn @ (v + bv) =
    # attn @ v + bv; bv then passes through proj as a constant vector.
    proj_b = np.asarray(proj_b, np.float32)
    qkv_b = np.asarray(qkv_b, np.float32)
    proj_w = np.asarray(proj_w, np.float32)
    bias_full = proj_b + qkv_b[2 * C:] @ proj_w
    outs = []
    for b in range(B):
        t0 = res.results[2 * b]["out"].reshape(C, N).astype(np.float32)
        t1 = res.results[2 * b + 1]["out"].reshape(C, N).astype(np.float32)
        outs.append((t0 + t1).T + bias_full)
    return np.stack(outs).reshape(B, D, H, W, C).astype(np.float32)


# revision 51
# speedup vs baseline: 1.2336x; 1.2336x over previous
"""Distributed Bass kernel: 3D windowed attention with decomposed rel-pos bias.

Sharding: 8 cores = 4 batches x 2 head-groups (6 heads each).
Per-core layout is fully transposed ([channel, token]); the rel-pos bias is
folded into the scores matmul as 36 extra contraction channels (one-hot
k-position rows in the stationary operand, F = q.R tables in the moving
operand).  Softmax runs max-free; the denominator comes free as a ones-row
appended to V in the AV matmul.  All matmuls run in bf16 with fp32 PSUM.

v2 restructure (from trace analysis of the 448us baseline):
- scores PSUM is a [128, 784] 2-bank tile; ONE 784-wide Exp ACT per
  (head, q-half, k-tile) instead of four 512/32-wide ones (scalar engine
  fixed cost is ~352cyc/instr; this cuts scalar time ~40%).
- softmax 1/denom via DVE reciprocal_approx_fast instead of scalar
  Ln+Exp: removes 12 ACT_TABLE_LOAD bounces (~32us of scalar).
- V projection bias folded into the host-side output bias (rows of a
  normalized softmax sum to 1), removing per-head Identity ACTs.
- next pair's QKV/F matmuls are dribbled into the attention k-loop every
  few k-tiles so the PE never idles (HAM clock-gate stays warm).
- aug assembly copies moved to DMA (q rows, one-hot rows) or merged
  (both heads per F copy, V copy via strided dest) to unload the DVE.
"""

import os
import sys

import numpy as np

sys.path.insert(0, "/opt/trn_rl_repo")

B, D, H, W, C = 4, 8, 14, 14, 768
NH, HD = 12, 64
N = D * H * W  # 1568
HPC = 6  # heads per core
SCALE = HD ** -0.5
NKC = C // 128  # 6 k-chunks of input channels
NKT = (N + 127) // 128  # 13 token tiles (12x128 + 32)
FCH = [(0, 512), (512, 512), (1024, 512), (1536, 32)]  # free-dim chunks of N
QHW = 784  # q columns per attention half
HCH = [(0, 512), (512, 272)]  # free-dim chunks of a 784 half
KAUG = 100  # 64 qk channels + 8 + 14 + 14 bias channels

_CACHED = {}


def _build_nc():
    import concourse.bass as bass  # noqa: F401
    import concourse.mybir as mybir
    import concourse.tile as tile
    from concourse import bacc

    f32 = mybir.dt.float32
    bf16 = mybir.dt.bfloat16
    AF = mybir.ActivationFunctionType

    nc = bacc.Bacc(None, target_bir_lowering=False)

    # --- DRAM parameters (per-core shards; host pre-transposes/reorders) ---
    xT_d = nc.declare_dram_parameter("xT", [NKC, 128, N], bf16, isOutput=False)
    wqkv_d = nc.declare_dram_parameter("wqkv", [NKC, 128, 1152], bf16, isOutput=False)
    wproj_d = nc.declare_dram_parameter("wproj", [3, 128, 768], bf16, isOutput=False)
    oneh_d = nc.declare_dram_parameter("oneh", [36, N], bf16, isOutput=False)
    rdT_d = nc.declare_dram_parameter("rdT", [128, D * 40], bf16, isOutput=False)
    rhT_d = nc.declare_dram_parameter("rhT", [128, H * 46], bf16, isOutput=False)
    rwT_d = nc.declare_dram_parameter("rwT", [128, W * 46], bf16, isOutput=False)
    bqk_d = nc.declare_dram_parameter("bqk", [128, 6], f32, isOutput=False)
    out_d = nc.declare_dram_parameter("out", [NKC, 128, N], bf16, isOutput=True)

    with tile.TileContext(nc) as tc:
        with (
            tc.tile_pool(name="const", bufs=1) as cpool,
            tc.tile_pool(name="work", bufs=2) as wpool,
            tc.tile_pool(name="qkp", bufs=2, space="PSUM") as qkp,
            tc.tile_pool(name="avp", bufs=1, space="PSUM") as avp,
            tc.tile_pool(name="mmp", bufs=2, space="PSUM") as mmp,
        ):
            # ---- load constants ----
            xT = cpool.tile([128, NKC * N], bf16)
            wqkv = cpool.tile([128, NKC * 1152], bf16)
            wproj = cpool.tile([128, 3 * 768], bf16)
            oneh = cpool.tile([36, N], bf16)
            rdT = cpool.tile([128, D * 40], bf16)
            rhT = cpool.tile([128, H * 46], bf16)
            rwT = cpool.tile([128, W * 46], bf16)
            bqk = cpool.tile([128, 6], f32)
            # xT/wqkv land first (first matmuls need all of both); wproj last
            for kc in range(NKC):
                nc.sync.dma_start(xT[:, kc * N:(kc + 1) * N], xT_d[kc])
                nc.sync.dma_start(wqkv[:, kc * 1152:(kc + 1) * 1152], wqkv_d[kc])
            nc.sync.dma_start(bqk[:], bqk_d[:])
            nc.sync.dma_start(rdT[:], rdT_d[:])
            nc.sync.dma_start(rhT[:], rhT_d[:])
            nc.sync.dma_start(rwT[:], rwT_d[:])
            nc.sync.dma_start(oneh[:], oneh_d[:])
            for t3 in range(3):
                nc.sync.dma_start(wproj[:, t3 * 768:(t3 + 1) * 768], wproj_d[t3])

            # ---- persistent SBUF state ----
            # V in natural [token, channel] layout, ones column per head
            vnat = cpool.tile([128, NKT, HPC * 65], bf16)
            av_all = [
                cpool.tile([128, N], bf16, name=f"av_all{i}", tag=f"av{i}")
                for i in range(3)
            ]

            # ---- deferred-work queue (dribbled into attention k-loops) ----
            pending = []

            def dribble():
                if pending:
                    pending.pop(0)()

            def emit_v(kt):
                kp = min(128, N - kt * 128)
                pv = mmp.tile([128, 512], f32, name="pv", tag="mm")
                for kc in range(NKC):
                    nc.tensor.matmul(
                        pv[0:kp, 0:384],
                        xT[:, kc * N + kt * 128: kc * N + kt * 128 + kp],
                        wqkv[:, kc * 1152 + 768: kc * 1152 + 1152],
                        start=(kc == 0), stop=(kc == NKC - 1),
                    )
                dst = vnat[0:kp, kt, :].rearrange("p (h c) -> p h c", h=HPC)
                src = pv[0:kp, 0:384].rearrange("p (h c) -> p h c", h=HPC)
                nc.vector.tensor_copy(dst[:, :, 0:64], src)

            # ---- QKV for one head pair (queued as closures) ----
            def emit_qkv(p, eager):
                augs = []
                for x in range(2):
                    q_t = wpool.tile([KAUG, N], bf16, name=f"qaug{x}", tag=f"qaug{x}")
                    k_t = wpool.tile([128, N], bf16, name=f"kaug{x}", tag=f"kaug{x}")
                    augs.append((q_t, k_t))
                qpair = wpool.tile([128, N], bf16, name="qpair", tag="qpair")

                def chunk(qk, f0, fl):
                    def run():
                        col0 = qk * 384 + p * 128
                        bcol = qk * 3 + p
                        ps = mmp.tile([128, 512], f32, name="ps", tag="mm")
                        for kc in range(NKC):
                            nc.tensor.matmul(
                                ps[:, 0:fl],
                                wqkv[:, kc * 1152 + col0: kc * 1152 + col0 + 128],
                                xT[:, kc * N + f0: kc * N + f0 + fl],
                                start=(kc == 0), stop=(kc == NKC - 1),
                            )
                        # pair 0's bias-adds ride the then-idle scalar engine
                        # so the DVE can assemble the aug tiles sooner
                        if qk == 0:
                            if eager:
                                nc.scalar.add(
                                    qpair[:, f0:f0 + fl], ps[:, 0:fl],
                                    bqk[:, bcol:bcol + 1],
                                )
                            else:
                                nc.vector.tensor_scalar_add(
                                    qpair[:, f0:f0 + fl], ps[:, 0:fl],
                                    bqk[:, bcol:bcol + 1],
                                )
                        else:
                            for x in range(2):
                                if eager:
                                    nc.scalar.add(
                                        augs[x][1][0:64, f0:f0 + fl],
                                        ps[x * 64:(x + 1) * 64, 0:fl],
                                        bqk[x * 64:(x + 1) * 64, bcol:bcol + 1],
                                    )
                                else:
                                    nc.vector.tensor_scalar_add(
                                        augs[x][1][0:64, f0:f0 + fl],
                                        ps[x * 64:(x + 1) * 64, 0:fl],
                                        bqk[x * 64:(x + 1) * 64, bcol:bcol + 1],
                                    )
                    return run

                def assemble():
                    for x in range(2):
                        nc.sync.dma_start(
                            augs[x][0][0:64, :], qpair[x * 64:(x + 1) * 64, :]
                        )
                        nc.sync.dma_start(augs[x][1][64:KAUG, :], oneh[:])

                jobs = [chunk(qk, f0, fl) for qk in range(2) for (f0, fl) in FCH]
                jobs.append(assemble)
                if eager:
                    for j in jobs:
                        j()
                else:
                    pending.extend(jobs)
                return augs, qpair

            # ---- F = q.R rel-pos tables for one pair (queued as closures) ----
            def emit_F(p, augs, qpair, eager):
                qpv = qpair.rearrange("p (d h w) -> p d h w", d=D, h=H, w=W)

                def rel_d():
                    fsD = wpool.tile([40, N], bf16, name="fsD", tag="fsD")
                    for qd in range(D):
                        pf = mmp.tile([128, 512], f32, name="pf", tag="mm")
                        nc.tensor.matmul(
                            pf[0:40, 0:H * W],
                            rdT[:, qd * 40:(qd + 1) * 40],
                            qpair[:, qd * H * W:(qd + 1) * H * W],
                        )
                        nc.vector.tensor_copy(
                            fsD[:, qd * H * W:(qd + 1) * H * W], pf[0:40, 0:H * W]
                        )
                    for x in range(2):
                        nc.sync.dma_start(augs[x][0][64:72, :], fsD[32 * x:32 * x + 8, :])

                def rel_h():
                    fsH = wpool.tile([46, N], bf16, name="fsH", tag="fsH")
                    fv = fsH.rearrange("p (d h w) -> p d h w", d=D, h=H, w=W)
                    for qh in range(H):
                        pf = mmp.tile([128, 512], f32, name="pf", tag="mm")
                        nc.tensor.matmul(
                            pf[0:46, 0:D * W], rhT[:, qh * 46:(qh + 1) * 46],
                            qpv[:, :, qh, :]
                        )
                        nc.vector.tensor_copy(fv[:, :, qh, :], pf[0:46, 0:D * W])
                    for x in range(2):
                        nc.sync.dma_start(augs[x][0][72:86, :], fsH[32 * x:32 * x + 14, :])

                def rel_w():
                    fsW = wpool.tile([46, N], bf16, name="fsW", tag="fsW")
                    fv = fsW.rearrange("p (d h w) -> p d h w", d=D, h=H, w=W)
                    for qw in range(W):
                        pf = mmp.tile([128, 512], f32, name="pf", tag="mm")
                        nc.tensor.matmul(
                            pf[0:46, 0:D * H], rwT[:, qw * 46:(qw + 1) * 46],
                            qpv[:, :, :, qw]
                        )
                        nc.vector.tensor_copy(fv[:, :, :, qw], pf[0:46, 0:D * H])
                    for x in range(2):
                        nc.sync.dma_start(augs[x][0][86:100, :], fsW[32 * x:32 * x + 14, :])

                jobs = [rel_d, rel_h, rel_w]
                if eager:
                    for j in jobs:
                        j()
                else:
                    pending.extend(jobs)

            # ---- attention for one head over one 784-wide q half ----
            def emit_att(p, x, qh, avst, denb, jit_v=False):
                h6 = 2 * p + x
                q_t, k_t = augs_cur[x]
                pav = avp.tile([65, QHW], f32, name="pav", tag="av")
                greedy = (p == 2 and x == 1 and qh == 1)
                for kt in range(NKT):
                    kp = min(128, N - kt * 128)
                    if jit_v:
                        emit_v(kt)
                    elif greedy or kt % 5 == 1:
                        dribble()
                    qk = qkp.tile([128, QHW], f32, name="qk", tag="qk")
                    for (f0, fl) in HCH:
                        nc.tensor.matmul(
                            qk[0:kp, f0:f0 + fl],
                            k_t[0:KAUG, kt * 128: kt * 128 + kp],
                            q_t[0:KAUG, qh * QHW + f0: qh * QHW + f0 + fl],
                        )
                    et = wpool.tile([128, QHW], bf16, name="et", tag="et", bufs=3)
                    nc.scalar.activation(et[0:kp, :], qk[0:kp, 0:QHW], AF.Exp)
                    for (f0, fl) in HCH:
                        nc.tensor.matmul(
                            pav[:, f0:f0 + fl],
                            vnat[0:kp, kt, h6 * 65:(h6 + 1) * 65],
                            et[0:kp, f0:f0 + fl],
                            start=(kt == 0), stop=(kt == NKT - 1),
                        )
                # drain av rows + denominator row; recip needs its input at
                # partition 0, so the denominator row gets its own copy
                nc.vector.tensor_copy(
                    avst[0:64, qh * QHW:(qh + 1) * QHW], pav[0:64, :]
                )
                nc.vector.tensor_copy(
                    denb[0:1, qh * QHW:(qh + 1) * QHW], pav[64:65, :]
                )
                dribble()

            # ---- normalize one q-half of one head into av_all ----
            def emit_norm(p, x, qh, avst, denb):
                sl = slice(qh * QHW, (qh + 1) * QHW)
                rcp = wpool.tile([1, QHW], f32, name="rcp", tag="rcp")
                rbc = wpool.tile([64, QHW], f32, name="rbc", tag="rbc")
                nc.vector.reciprocal_approx_fast(rcp[0:1, :], denb[0:1, sl])
                nc.gpsimd.partition_broadcast(rbc[0:64, :], rcp[0:1, :])
                nc.vector.tensor_mul(
                    av_all[p][x * 64:(x + 1) * 64, sl], avst[0:64, sl], rbc[0:64, :]
                )

            # ---- projection helpers ----
            ots = [
                cpool.tile([128, N], bf16, name=f"ot{mo}", tag=f"out{mo}")
                for mo in range(NKC)
            ]

            def proj_chunk(mo, f0, fl, alt=0):
                def run():
                    # alternate PSUM pool and copy engine for a deeper
                    # tail pipeline (avp's banks are free by proj time)
                    pool = mmp if alt % 2 == 0 else avp
                    tag = "mm" if alt % 2 == 0 else "av"
                    ps = pool.tile([128, 512], f32, name="ps", tag=tag)
                    for t3 in range(3):
                        nc.tensor.matmul(
                            ps[:, 0:fl],
                            wproj[:, t3 * 768 + mo * 128: t3 * 768 + mo * 128 + 128],
                            av_all[t3][:, f0:f0 + fl],
                            start=(t3 == 0), stop=(t3 == 2),
                        )
                    if alt % 2 == 0:
                        nc.vector.tensor_copy(ots[mo][:, f0:f0 + fl], ps[:, 0:fl])
                    else:
                        nc.scalar.copy(ots[mo][:, f0:f0 + fl], ps[:, 0:fl])
                return run

            # ---- pipeline driver ----
            augs_cur, qpair_cur = emit_qkv(0, eager=True)
            emit_F(0, augs_cur, qpair_cur, eager=True)
            nc.vector.memset(vnat[:], 1.0)
            for p in range(3):
                avsts = [
                    wpool.tile([64, N], f32, name=f"avst{x}", tag=f"avst{x}", bufs=1)
                    for x in range(2)
                ]
                denbs = [
                    wpool.tile([1, N], f32, name=f"denb{x}", tag=f"denb{x}", bufs=1)
                    for x in range(2)
                ]
                if p < 2:
                    augs_nxt, qpair_nxt = emit_qkv(p + 1, eager=False)
                    emit_F(p + 1, augs_nxt, qpair_nxt, eager=False)
                for x in range(2):
                    for qh in range(2):
                        if p == 2 and x == 1 and qh == 1:
                            # proj cols 0:784 only need the qh=0 halves.
                            # alt=0 only: avp's slot is still held by pav here.
                            pending.extend(
                                proj_chunk(mo, f0, fl, alt=0)
                                for mo in range(NKC)
                                for (f0, fl) in ((0, 512), (512, 272))
                            )
                        emit_att(p, x, qh, avsts[x], denbs[x],
                                 jit_v=(p == 0 and x == 0 and qh == 0))
                        emit_norm(p, x, qh, avsts[x], denbs[x])
                if p < 2:
                    while pending:
                        dribble()
                    augs_cur, qpair_cur = augs_nxt, qpair_nxt

            # ---- rest of the partial projection: outT[768, N] ----
            while pending:
                dribble()
            alt = 0
            for mo in range(NKC):
                for (f0, fl) in ((784, 240), (1024, 512), (1536, 32)):
                    proj_chunk(mo, f0, fl, alt=alt)()
                    alt += 1
                nc.sync.dma_start(out_d[mo], ots[mo][:])

    nc.compile()
    return nc


def _prep_inputs(x, qkv_w, qkv_b, proj_w, proj_b, rel_pos_d, rel_pos_h, rel_pos_w):
    """Host-side shard prep: returns in_maps list for 8 cores."""
    import ml_dtypes
    bf = ml_dtypes.bfloat16
    x = np.ascontiguousarray(x, np.float32)
    qkv_w = np.asarray(qkv_w, np.float32)
    qkv_b = np.asarray(qkv_b, np.float32)
    proj_w = np.asarray(proj_w, np.float32)

    # one-hot k-position rows [36, N]
    j = np.arange(N)
    kd, kh, kw = j // (H * W), (j // W) % H, j % W
    oneh = np.zeros((36, N), np.float32)
    oneh[kd, j] = 1.0
    oneh[8 + kh, j] = 1.0
    oneh[22 + kw, j] = 1.0
    oneh = oneh.astype(bf)

    # rel tables, transposed and un-scaled (q is pre-scaled by SCALE).
    # Block-diagonal over the head pair: head A channels in rows 0:64 feed
    # psum rows 0:n, head B channels in rows 64:128 feed psum rows 32:32+n.
    def rtab(table, n, span):
        t = np.asarray(table, np.float32) / SCALE  # [2n-1, 64]
        qq, kk = np.meshgrid(np.arange(n), np.arange(n), indexing="ij")
        base = t[(qq - kk + n - 1).reshape(-1)].T.reshape(64, n, n)  # [c, q, k]
        out = np.zeros((128, n, span), np.float32)
        out[0:64, :, 0:n] = base
        out[64:128, :, 32:32 + n] = base
        return np.ascontiguousarray(out.reshape(128, n * span)).astype(bf)

    rdT = rtab(rel_pos_d, D, 40)
    rhT = rtab(rel_pos_h, H, 46)
    rwT = rtab(rel_pos_w, W, 46)

    in_maps = []
    for core in range(8):
        b, g = divmod(core, 2)
        heads = list(range(g * HPC, (g + 1) * HPC))
        # W columns: [q(6x64) | k(6x64) | v(6x64)] for this head group; q scaled
        cols_q = [0 * C + h * HD + c for h in heads for c in range(HD)]
        cols_k = [1 * C + h * HD + c for h in heads for c in range(HD)]
        cols_v = [2 * C + h * HD + c for h in heads for c in range(HD)]
        wq = qkv_w[:, cols_q] * SCALE
        wk = qkv_w[:, cols_k]
        wv = qkv_w[:, cols_v]
        wc = np.concatenate([wq, wk, wv], axis=1)  # [768, 1152]
        wqkv = np.ascontiguousarray(wc.reshape(NKC, 128, 1152)).astype(bf)

        bq = qkv_b[cols_q] * SCALE
        bk = qkv_b[cols_k]
        bqk = np.zeros((128, 6), np.float32)
        for p in range(3):
            bqk[:, p] = bq[p * 128:(p + 1) * 128]
            bqk[:, 3 + p] = bk[p * 128:(p + 1) * 128]

        rows = [h * HD + c for h in heads for c in range(HD)]
        wp = np.ascontiguousarray(proj_w[rows].reshape(3, 128, 768)).astype(bf)

        xT = np.ascontiguousarray(
            x[b].reshape(N, C).T.reshape(NKC, 128, N)
        ).astype(bf)
        in_maps.append({
            "xT": xT, "wqkv": wqkv, "wproj": wp, "oneh": oneh,
            "rdT": rdT, "rhT": rhT, "rwT": rwT, "bqk": bqk,
        })
    return in_maps


def _install_ntff_hook_shim():
    """The image's antenv package lacks axon_hooks; recreate it so
    run_bass_kernel_spmd(trace=True) can reach the libaxon NTFF profiler."""
    import types

    if "antenv.axon_hooks" in sys.modules:
        return
    import antenv
    mod = types.ModuleType("antenv.axon_hooks")
    _hook = [None]
    mod.set_axon_ntff_profile_hook = lambda h: _hook.__setitem__(0, h)
    mod.get_axon_ntff_profile_hook = lambda: _hook[0]
    antenv.axon_hooks = mod
    sys.modules["antenv.axon_hooks"] = mod
    try:
        from trn_agent_boot.trn_boot import _ntff_profile_via_ctypes

        mod.set_axon_ntff_profile_hook(
            _ntff_profile_via_ctypes("/opt/axon/libaxon_pjrt.so")
        )
    except Exception as e:  # degrade to no tracing
        print(f"ntff hook shim failed: {e}", file=sys.stderr)


def _patch_ldw_opt():
    """Enable walrus's LDWEIGHTS dedup (consecutive matmuls reusing the same
    stationary operand skip the reload). bass_utils hardcodes it off.
    Disabled: walrus codegen fails with it on for this BIR."""
    if not bool(int(os.environ.get("KERNEL_LDW_OPT", "0"))):
        return
    import concourse.bass_utils as bu

    if getattr(bu, "_ldw_patched", False):
        return
    orig = bu.run_command

    def run_command(cmd, *a, **kw):
        if isinstance(cmd, list):
            cmd = [
                "--enable-ldw-opt=true" if c == "--enable-ldw-opt=false" else c
                for c in cmd
            ]
        return orig(cmd, *a, **kw)

    bu.run_command = run_command
    bu._ldw_patched = True


LAST_EXEC_NS = None


def kernel(x, qkv_w, qkv_b, proj_w, proj_b, rel_pos_d, rel_pos_h, rel_pos_w):
    global LAST_EXEC_NS
    if "nc" not in _CACHED:
        _CACHED["nc"] = _build_nc()
    nc = _CACHED["nc"]
    in_maps = _prep_inputs(
        x, qkv_w, qkv_b, proj_w, proj_b, rel_pos_d, rel_pos_h, rel_pos_w
    )
    _patch_ldw_opt()
    from concourse.bass_utils import run_bass_kernel_spmd

    trace = bool(int(os.environ.get("KERNEL_TRACE", "0")))
    if trace:
        _install_ntff_hook_shim()
    res = run_bass_kernel_spmd(nc, in_maps, core_ids=list(range(8)), trace=trace)
    LAST_EXEC_NS = res.exec_time_ns
    # V-projection bias: softmax rows sum to 1, so attn @ (v + bv) =
    # attn @ v + bv; bv then passes through proj as a constant vector.
    proj_b = np.asarray(proj_b, np.float32)
    qkv_b = np.asarray(qkv_b, np.float32)
    proj_w = np.asarray(proj_w, np.float32)
    bias_full = proj_b + qkv_b[2 * C:] @ proj_w
    outs = []
    for b in range(B):
        t0 = res.results[2 * b]["out"].reshape(C, N).astype(np.float32)
        t1 = res.results[2 * b + 1]["out"].reshape(C, N).astype(np.float32)
        outs.append((t0 + t1).T + bias_full)
    return np.stack(outs).reshape(B, D, H, W, C).astype(np.float32)


# revision 55
# speedup vs baseline: 1.2392x; 1.0046x over previous
"""Distributed Bass kernel: 3D windowed attention with decomposed rel-pos bias.

Sharding: 8 cores = 4 batches x 2 head-groups (6 heads each).
Per-core layout is fully transposed ([channel, token]); the rel-pos bias is
folded into the scores matmul as 36 extra contraction channels (one-hot
k-position rows in the stationary operand, F = q.R tables in the moving
operand).  Softmax runs max-free; the denominator comes free as a ones-row
appended to V in the AV matmul.  All matmuls run in bf16 with fp32 PSUM.

v2 restructure (from trace analysis of the 448us baseline):
- scores PSUM is a [128, 784] 2-bank tile; ONE 784-wide Exp ACT per
  (head, q-half, k-tile) instead of four 512/32-wide ones (scalar engine
  fixed cost is ~352cyc/instr; this cuts scalar time ~40%).
- softmax 1/denom via DVE reciprocal_approx_fast instead of scalar
  Ln+Exp: removes 12 ACT_TABLE_LOAD bounces (~32us of scalar).
- V projection bias folded into the host-side output bias (rows of a
  normalized softmax sum to 1), removing per-head Identity ACTs.
- next pair's QKV/F matmuls are dribbled into the attention k-loop every
  few k-tiles so the PE never idles (HAM clock-gate stays warm).
- aug assembly copies moved to DMA (q rows, one-hot rows) or merged
  (both heads per F copy, V copy via strided dest) to unload the DVE.
"""

import os
import sys

import numpy as np

sys.path.insert(0, "/opt/trn_rl_repo")

B, D, H, W, C = 4, 8, 14, 14, 768
NH, HD = 12, 64
N = D * H * W  # 1568
HPC = 6  # heads per core
SCALE = HD ** -0.5
NKC = C // 128  # 6 k-chunks of input channels
NKT = (N + 127) // 128  # 13 token tiles (12x128 + 32)
FCH = [(0, 512), (512, 512), (1024, 512), (1536, 32)]  # free-dim chunks of N
QHW = 784  # q columns per attention half
HCH = [(0, 512), (512, 272)]  # free-dim chunks of a 784 half
KAUG = 100  # 64 qk channels + 8 + 14 + 14 bias channels

_CACHED = {}


def _build_nc():
    import concourse.bass as bass  # noqa: F401
    import concourse.mybir as mybir
    import concourse.tile as tile
    from concourse import bacc

    f32 = mybir.dt.float32
    bf16 = mybir.dt.bfloat16
    AF = mybir.ActivationFunctionType

    nc = bacc.Bacc(None, target_bir_lowering=False)

    # --- DRAM parameters (per-core shards; host pre-transposes/reorders) ---
    xT_d = nc.declare_dram_parameter("xT", [NKC, 128, N], bf16, isOutput=False)
    wqkv_d = nc.declare_dram_parameter("wqkv", [NKC, 128, 1152], bf16, isOutput=False)
    wproj_d = nc.declare_dram_parameter("wproj", [3, 128, 768], bf16, isOutput=False)
    oneh_d = nc.declare_dram_parameter("oneh", [36, N], bf16, isOutput=False)
    rdT_d = nc.declare_dram_parameter("rdT", [128, D * 40], bf16, isOutput=False)
    rhT_d = nc.declare_dram_parameter("rhT", [128, H * 46], bf16, isOutput=False)
    rwT_d = nc.declare_dram_parameter("rwT", [128, W * 46], bf16, isOutput=False)
    bqk_d = nc.declare_dram_parameter("bqk", [128, 6], f32, isOutput=False)
    out_d = nc.declare_dram_parameter("out", [NKC, 128, N], bf16, isOutput=True)

    with tile.TileContext(nc) as tc:
        with (
            tc.tile_pool(name="const", bufs=1) as cpool,
            tc.tile_pool(name="work", bufs=2) as wpool,
            tc.tile_pool(name="qkp", bufs=2, space="PSUM") as qkp,
            tc.tile_pool(name="avp", bufs=1, space="PSUM") as avp,
            tc.tile_pool(name="mmp", bufs=2, space="PSUM") as mmp,
        ):
            # ---- load constants ----
            xT = cpool.tile([128, NKC * N], bf16)
            wqkv = cpool.tile([128, NKC * 1152], bf16)
            wproj = cpool.tile([128, 3 * 768], bf16)
            oneh = cpool.tile([36, N], bf16)
            rdT = cpool.tile([128, D * 40], bf16)
            rhT = cpool.tile([128, H * 46], bf16)
            rwT = cpool.tile([128, W * 46], bf16)
            bqk = cpool.tile([128, 6], f32)
            # xT/wqkv land first (first matmuls need all of both); wproj last
            for kc in range(NKC):
                nc.sync.dma_start(xT[:, kc * N:(kc + 1) * N], xT_d[kc])
                nc.sync.dma_start(wqkv[:, kc * 1152:(kc + 1) * 1152], wqkv_d[kc])
            nc.sync.dma_start(bqk[:], bqk_d[:])
            nc.sync.dma_start(rdT[:], rdT_d[:])
            nc.sync.dma_start(rhT[:], rhT_d[:])
            nc.sync.dma_start(rwT[:], rwT_d[:])
            nc.sync.dma_start(oneh[:], oneh_d[:])
            for t3 in range(3):
                nc.sync.dma_start(wproj[:, t3 * 768:(t3 + 1) * 768], wproj_d[t3])

            # ---- persistent SBUF state ----
            # V in natural [token, channel] layout, ones column per head
            vnat = cpool.tile([128, NKT, HPC * 65], bf16)
            av_all = [
                cpool.tile([128, N], bf16, name=f"av_all{i}", tag=f"av{i}")
                for i in range(3)
            ]

            # ---- deferred-work queue (dribbled into attention k-loops) ----
            pending = []

            def dribble():
                if pending:
                    pending.pop(0)()

            def emit_v(kt):
                kp = min(128, N - kt * 128)
                pv = mmp.tile([128, 512], f32, name="pv", tag="mm")
                for kc in range(NKC):
                    nc.tensor.matmul(
                        pv[0:kp, 0:384],
                        xT[:, kc * N + kt * 128: kc * N + kt * 128 + kp],
                        wqkv[:, kc * 1152 + 768: kc * 1152 + 1152],
                        start=(kc == 0), stop=(kc == NKC - 1),
                    )
                dst = vnat[0:kp, kt, :].rearrange("p (h c) -> p h c", h=HPC)
                src = pv[0:kp, 0:384].rearrange("p (h c) -> p h c", h=HPC)
                nc.vector.tensor_copy(dst[:, :, 0:64], src)

            # ---- QKV for one head pair (queued as closures) ----
            def emit_qkv(p, eager):
                augs = []
                for x in range(2):
                    q_t = wpool.tile([KAUG, N], bf16, name=f"qaug{x}", tag=f"qaug{x}")
                    k_t = wpool.tile([128, N], bf16, name=f"kaug{x}", tag=f"kaug{x}")
                    augs.append((q_t, k_t))
                qpair = wpool.tile([128, N], bf16, name="qpair", tag="qpair")

                def chunk(qk, f0, fl):
                    def run():
                        col0 = qk * 384 + p * 128
                        bcol = qk * 3 + p
                        ps = mmp.tile([128, 512], f32, name="ps", tag="mm")
                        for kc in range(NKC):
                            nc.tensor.matmul(
                                ps[:, 0:fl],
                                wqkv[:, kc * 1152 + col0: kc * 1152 + col0 + 128],
                                xT[:, kc * N + f0: kc * N + f0 + fl],
                                start=(kc == 0), stop=(kc == NKC - 1),
                            )
                        # pair 0's bias-adds ride the then-idle scalar engine
                        # so the DVE can assemble the aug tiles sooner
                        if qk == 0:
                            if eager:
                                nc.scalar.add(
                                    qpair[:, f0:f0 + fl], ps[:, 0:fl],
                                    bqk[:, bcol:bcol + 1],
                                )
                            else:
                                nc.vector.tensor_scalar_add(
                                    qpair[:, f0:f0 + fl], ps[:, 0:fl],
                                    bqk[:, bcol:bcol + 1],
                                )
                        else:
                            for x in range(2):
                                if eager:
                                    nc.scalar.add(
                                        augs[x][1][0:64, f0:f0 + fl],
                                        ps[x * 64:(x + 1) * 64, 0:fl],
                                        bqk[x * 64:(x + 1) * 64, bcol:bcol + 1],
                                    )
                                else:
                                    nc.vector.tensor_scalar_add(
                                        augs[x][1][0:64, f0:f0 + fl],
                                        ps[x * 64:(x + 1) * 64, 0:fl],
                                        bqk[x * 64:(x + 1) * 64, bcol:bcol + 1],
                                    )
                    return run

                def assemble():
                    for x in range(2):
                        nc.sync.dma_start(
                            augs[x][0][0:64, :], qpair[x * 64:(x + 1) * 64, :]
                        )
                        nc.sync.dma_start(augs[x][1][64:KAUG, :], oneh[:])

                jobs = [chunk(qk, f0, fl) for qk in range(2) for (f0, fl) in FCH]
                jobs.append(assemble)
                if eager:
                    for j in jobs:
                        j()
                else:
                    pending.extend(jobs)
                return augs, qpair

            # ---- F = q.R rel-pos tables for one pair (queued as closures) ----
            def emit_F(p, augs, qpair, eager):
                qpv = qpair.rearrange("p (d h w) -> p d h w", d=D, h=H, w=W)

                def rel_d():
                    fsD = wpool.tile([40, N], bf16, name="fsD", tag="fsD")
                    for qd in range(D):
                        pf = mmp.tile([128, 512], f32, name="pf", tag="mm")
                        nc.tensor.matmul(
                            pf[0:40, 0:H * W],
                            rdT[:, qd * 40:(qd + 1) * 40],
                            qpair[:, qd * H * W:(qd + 1) * H * W],
                        )
                        nc.vector.tensor_copy(
                            fsD[:, qd * H * W:(qd + 1) * H * W], pf[0:40, 0:H * W]
                        )
                    for x in range(2):
                        nc.sync.dma_start(augs[x][0][64:72, :], fsD[32 * x:32 * x + 8, :])

                def rel_h():
                    fsH = wpool.tile([46, N], bf16, name="fsH", tag="fsH")
                    fv = fsH.rearrange("p (d h w) -> p d h w", d=D, h=H, w=W)
                    for qh in range(H):
                        pf = mmp.tile([128, 512], f32, name="pf", tag="mm")
                        nc.tensor.matmul(
                            pf[0:46, 0:D * W], rhT[:, qh * 46:(qh + 1) * 46],
                            qpv[:, :, qh, :]
                        )
                        nc.vector.tensor_copy(fv[:, :, qh, :], pf[0:46, 0:D * W])
                    for x in range(2):
                        nc.sync.dma_start(augs[x][0][72:86, :], fsH[32 * x:32 * x + 14, :])

                def rel_w():
                    fsW = wpool.tile([46, N], bf16, name="fsW", tag="fsW")
                    fv = fsW.rearrange("p (d h w) -> p d h w", d=D, h=H, w=W)
                    for qw in range(W):
                        pf = mmp.tile([128, 512], f32, name="pf", tag="mm")
                        nc.tensor.matmul(
                            pf[0:46, 0:D * H], rwT[:, qw * 46:(qw + 1) * 46],
                            qpv[:, :, :, qw]
                        )
                        nc.vector.tensor_copy(fv[:, :, :, qw], pf[0:46, 0:D * H])
                    for x in range(2):
                        nc.sync.dma_start(augs[x][0][86:100, :], fsW[32 * x:32 * x + 14, :])

                jobs = [rel_d, rel_h, rel_w]
                if eager:
                    for j in jobs:
                        j()
                else:
                    pending.extend(jobs)

            # ---- attention for one head over one 784-wide q half ----
            # Attention is emitted as 12 "segments" (head x q-half), each a
            # 13-iteration k-loop, software-pipelined ACROSS segments: the AV
            # matmuls lag the QK matmuls by LAG k-tiles, and each segment's
            # first LAG QK+exp pairs are emitted before the previous
            # segment's tail AVs + drain.  Without this, the next segment's
            # first exp sits behind the previous segment's last AV (which
            # itself waits on an exp) in the PE's strict FIFO, stalling the
            # scalar engine ~1.5us at every one of the 24 boundaries.
            LAG = 2

            class Seg:
                def __init__(self, p, x, qh, avst, denb, jit_v):
                    self.p, self.x, self.qh = p, x, qh
                    self.avst, self.denb, self.jit_v = avst, denb, jit_v
                    self.h6 = 2 * p + x
                    self.q_t, self.k_t = augs_cur[x]
                    self.ets = {}
                    self.pav = None

                def qk_exp(self, kt):
                    kp = min(128, N - kt * 128)
                    if self.jit_v:
                        emit_v(kt)
                    elif kt % 5 == 1 or (self.p == 2 and self.x == 1 and self.qh == 1):
                        dribble()
                    qk = qkp.tile([128, QHW], f32, name="qk", tag="qk")
                    for (f0, fl) in HCH:
                        nc.tensor.matmul(
                            qk[0:kp, f0:f0 + fl],
                            self.k_t[0:KAUG, kt * 128: kt * 128 + kp],
                            self.q_t[0:KAUG, self.qh * QHW + f0:
                                     self.qh * QHW + f0 + fl],
                        )
                    et = wpool.tile([128, QHW], bf16, name="et", tag="et", bufs=5)
                    nc.scalar.activation(et[0:kp, :], qk[0:kp, 0:QHW], AF.Exp)
                    self.ets[kt] = et

                def av(self, kt):
                    if self.pav is None:
                        self.pav = avp.tile([65, QHW], f32, name="pav", tag="av")
                    kp = min(128, N - kt * 128)
                    et = self.ets.pop(kt)
                    for (f0, fl) in HCH:
                        nc.tensor.matmul(
                            self.pav[:, f0:f0 + fl],
                            vnat[0:kp, kt, self.h6 * 65:(self.h6 + 1) * 65],
                            et[0:kp, f0:f0 + fl],
                            start=(kt == 0), stop=(kt == NKT - 1),
                        )

                def begin(self):
                    for kt in range(LAG):
                        self.qk_exp(kt)

                def body(self):
                    for kt in range(LAG, NKT):
                        self.qk_exp(kt)
                        self.av(kt - LAG)

                def finish(self):
                    for kt in range(NKT - LAG, NKT):
                        self.av(kt)
                    sl = slice(self.qh * QHW, (self.qh + 1) * QHW)
                    nc.vector.tensor_copy(self.avst[0:64, sl], self.pav[0:64, :])
                    nc.vector.tensor_copy(self.denb[0:1, sl], self.pav[64:65, :])
                    dribble()
                    emit_norm(self.p, self.x, self.qh, self.avst, self.denb)

            # ---- normalize one q-half of one head into av_all ----
            def emit_norm(p, x, qh, avst, denb):
                sl = slice(qh * QHW, (qh + 1) * QHW)
                rcp = wpool.tile([1, QHW], f32, name="rcp", tag="rcp")
                rbc = wpool.tile([64, QHW], f32, name="rbc", tag="rbc")
                nc.vector.reciprocal_approx_fast(rcp[0:1, :], denb[0:1, sl])
                nc.gpsimd.partition_broadcast(rbc[0:64, :], rcp[0:1, :])
                nc.vector.tensor_mul(
                    av_all[p][x * 64:(x + 1) * 64, sl], avst[0:64, sl], rbc[0:64, :]
                )

            # ---- projection helpers ----
            ots = [
                cpool.tile([128, N], bf16, name=f"ot{mo}", tag=f"out{mo}")
                for mo in range(NKC)
            ]

            def proj_chunk(mo, f0, fl, alt=0):
                def run():
                    # alternate PSUM pool and copy engine for a deeper
                    # tail pipeline (avp's banks are free by proj time)
                    pool = mmp if alt % 2 == 0 else avp
                    tag = "mm" if alt % 2 == 0 else "av"
                    ps = pool.tile([128, 512], f32, name="ps", tag=tag)
                    for t3 in range(3):
                        nc.tensor.matmul(
                            ps[:, 0:fl],
                            wproj[:, t3 * 768 + mo * 128: t3 * 768 + mo * 128 + 128],
                            av_all[t3][:, f0:f0 + fl],
                            start=(t3 == 0), stop=(t3 == 2),
                        )
                    if alt % 2 == 0:
                        nc.vector.tensor_copy(ots[mo][:, f0:f0 + fl], ps[:, 0:fl])
                    else:
                        nc.scalar.copy(ots[mo][:, f0:f0 + fl], ps[:, 0:fl])
                return run

            # ---- pipeline driver ----
            augs_cur, qpair_cur = emit_qkv(0, eager=True)
            emit_F(0, augs_cur, qpair_cur, eager=True)
            nc.vector.memset(vnat[:], 1.0)
            prev_seg = None
            for p in range(3):
                avsts = [
                    wpool.tile([64, N], f32, name=f"avst{x}", tag=f"avst{x}", bufs=1)
                    for x in range(2)
                ]
                denbs = [
                    wpool.tile([1, N], f32, name=f"denb{x}", tag=f"denb{x}", bufs=1)
                    for x in range(2)
                ]
                if p > 0:
                    augs_cur, qpair_cur = augs_nxt, qpair_nxt
                if p < 2:
                    augs_nxt, qpair_nxt = emit_qkv(p + 1, eager=False)
                    emit_F(p + 1, augs_nxt, qpair_nxt, eager=False)
                for x in range(2):
                    for qh in range(2):
                        seg = Seg(p, x, qh, avsts[x], denbs[x],
                                  jit_v=(p == 0 and x == 0 and qh == 0))
                        seg.begin()
                        if prev_seg is not None:
                            prev_seg.finish()
                        if p == 2 and x == 1 and qh == 1:
                            # proj cols 0:784 only need the qh=0 halves;
                            # pushed after (2,1,0)'s norm is emitted.
                            # alt=0 only: avp's slot is still held by pav.
                            pending.extend(
                                proj_chunk(mo, f0, fl, alt=0)
                                for mo in range(NKC)
                                for (f0, fl) in ((0, 512), (512, 272))
                            )
                        seg.body()
                        prev_seg = seg
                # pair boundary: the next pair's aug tiles must be fully
                # assembled (dribbled closures run) before its first QK
                if p < 2:
                    while pending:
                        dribble()
            prev_seg.finish()

            # ---- rest of the partial projection: outT[768, N] ----
            while pending:
                dribble()
            alt = 0
            for mo in range(NKC):
                for (f0, fl) in ((784, 240), (1024, 512), (1536, 32)):
                    proj_chunk(mo, f0, fl, alt=alt)()
                    alt += 1
                nc.sync.dma_start(out_d[mo], ots[mo][:])

    nc.compile()
    return nc


def _prep_inputs(x, qkv_w, qkv_b, proj_w, proj_b, rel_pos_d, rel_pos_h, rel_pos_w):
    """Host-side shard prep: returns in_maps list for 8 cores."""
    import ml_dtypes
    bf = ml_dtypes.bfloat16
    x = np.ascontiguousarray(x, np.float32)
    qkv_w = np.asarray(qkv_w, np.float32)
    qkv_b = np.asarray(qkv_b, np.float32)
    proj_w = np.asarray(proj_w, np.float32)

    # one-hot k-position rows [36, N]
    j = np.arange(N)
    kd, kh, kw = j // (H * W), (j // W) % H, j % W
    oneh = np.zeros((36, N), np.float32)
    oneh[kd, j] = 1.0
    oneh[8 + kh, j] = 1.0
    oneh[22 + kw, j] = 1.0
    oneh = oneh.astype(bf)

    # rel tables, transposed and un-scaled (q is pre-scaled by SCALE).
    # Block-diagonal over the head pair: head A channels in rows 0:64 feed
    # psum rows 0:n, head B channels in rows 64:128 feed psum rows 32:32+n.
    def rtab(table, n, span):
        t = np.asarray(table, np.float32) / SCALE  # [2n-1, 64]
        qq, kk = np.meshgrid(np.arange(n), np.arange(n), indexing="ij")
        base = t[(qq - kk + n - 1).reshape(-1)].T.reshape(64, n, n)  # [c, q, k]
        out = np.zeros((128, n, span), np.float32)
        out[0:64, :, 0:n] = base
        out[64:128, :, 32:32 + n] = base
        return np.ascontiguousarray(out.reshape(128, n * span)).astype(bf)

    rdT = rtab(rel_pos_d, D, 40)
    rhT = rtab(rel_pos_h, H, 46)
    rwT = rtab(rel_pos_w, W, 46)

    in_maps = []
    for core in range(8):
        b, g = divmod(core, 2)
        heads = list(range(g * HPC, (g + 1) * HPC))
        # W columns: [q(6x64) | k(6x64) | v(6x64)] for this head group; q scaled
        cols_q = [0 * C + h * HD + c for h in heads for c in range(HD)]
        cols_k = [1 * C + h * HD + c for h in heads for c in range(HD)]
        cols_v = [2 * C + h * HD + c for h in heads for c in range(HD)]
        wq = qkv_w[:, cols_q] * SCALE
        wk = qkv_w[:, cols_k]
        wv = qkv_w[:, cols_v]
        wc = np.concatenate([wq, wk, wv], axis=1)  # [768, 1152]
        wqkv = np.ascontiguousarray(wc.reshape(NKC, 128, 1152)).astype(bf)

        bq = qkv_b[cols_q] * SCALE
        bk = qkv_b[cols_k]
        bqk = np.zeros((128, 6), np.float32)
        for p in range(3):
            bqk[:, p] = bq[p * 128:(p + 1) * 128]
            bqk[:, 3 + p] = bk[p * 128:(p + 1) * 128]

        rows = [h * HD + c for h in heads for c in range(HD)]
        wp = np.ascontiguousarray(proj_w[rows].reshape(3, 128, 768)).astype(bf)

        xT = np.ascontiguousarray(
            x[b].reshape(N, C).T.reshape(NKC, 128, N)
        ).astype(bf)
        in_maps.append({
            "xT": xT, "wqkv": wqkv, "wproj": wp, "oneh": oneh,
            "rdT": rdT, "rhT": rhT, "rwT": rwT, "bqk": bqk,
        })
    return in_maps


def _install_ntff_hook_shim():
    """The image's antenv package lacks axon_hooks; recreate it so
    run_bass_kernel_spmd(trace=True) can reach the libaxon NTFF profiler."""
    import types

    if "antenv.axon_hooks" in sys.modules:
        return
    import antenv
    mod = types.ModuleType("antenv.axon_hooks")
    _hook = [None]
    mod.set_axon_ntff_profile_hook = lambda h: _hook.__setitem__(0, h)
    mod.get_axon_ntff_profile_hook = lambda: _hook[0]
    antenv.axon_hooks = mod
    sys.modules["antenv.axon_hooks"] = mod
    try:
        from trn_agent_boot.trn_boot import _ntff_profile_via_ctypes

        mod.set_axon_ntff_profile_hook(
            _ntff_profile_via_ctypes("/opt/axon/libaxon_pjrt.so")
        )
    except Exception as e:  # degrade to no tracing
        print(f"ntff hook shim failed: {e}", file=sys.stderr)


def _patch_ldw_opt():
    """Enable walrus's LDWEIGHTS dedup (consecutive matmuls reusing the same
    stationary operand skip the reload). bass_utils hardcodes it off.
    Disabled: walrus codegen fails with it on for this BIR."""
    if not bool(int(os.environ.get("KERNEL_LDW_OPT", "0"))):
        return
    import concourse.bass_utils as bu

    if getattr(bu, "_ldw_patched", False):
        return
    orig = bu.run_command

    def run_command(cmd, *a, **kw):
        if isinstance(cmd, list):
            cmd = [
                "--enable-ldw-opt=true" if c == "--enable-ldw-opt=false" else c
                for c in cmd
            ]
        return orig(cmd, *a, **kw)

    bu.run_command = run_command
    bu._ldw_patched = True


LAST_EXEC_NS = None


def kernel(x, qkv_w, qkv_b, proj_w, proj_b, rel_pos_d, rel_pos_h, rel_pos_w):
    global LAST_EXEC_NS
    if "nc" not in _CACHED:
        _CACHED["nc"] = _build_nc()
    nc = _CACHED["nc"]
    in_maps = _prep_inputs(
        x, qkv_w, qkv_b, proj_w, proj_b, rel_pos_d, rel_pos_h, rel_pos_w
    )
    _patch_ldw_opt()
    from concourse.bass_utils import run_bass_kernel_spmd

    trace = bool(int(os.environ.get("KERNEL_TRACE", "0")))
    if trace:
        _install_ntff_hook_shim()
    res = run_bass_kernel_spmd(nc, in_maps, core_ids=list(range(8)), trace=trace)
    LAST_EXEC_NS = res.exec_time_ns
    # V-projection bias: softmax rows sum to 1, so attn @ (v + bv) =
    # attn @ v + bv; bv then passes through proj as a constant vector.
    proj_b = np.asarray(proj_b, np.float32)
    qkv_b = np.asarray(qkv_b, np.float32)
    proj_w = np.asarray(proj_w, np.float32)
    bias_full = proj_b + qkv_b[2 * C:] @ proj_w
    outs = []
    for b in range(B):
        t0 = res.results[2 * b]["out"].reshape(C, N).astype(np.float32)
        t1 = res.results[2 * b + 1]["out"].reshape(C, N).astype(np.float32)
        outs.append((t0 + t1).T + bias_full)
    return np.stack(outs).reshape(B, D, H, W, C).astype(np.float32)


# revision 58
# speedup vs baseline: 1.2718x; 1.0263x over previous
"""Distributed Bass kernel: 3D windowed attention with decomposed rel-pos bias.

Sharding: 8 cores = 4 batches x 2 head-groups (6 heads each).
Per-core layout is fully transposed ([channel, token]); the rel-pos bias is
folded into the scores matmul as 36 extra contraction channels (one-hot
k-position rows in the stationary operand, F = q.R tables in the moving
operand).  Softmax runs max-free; the denominator comes free as a ones-row
appended to V in the AV matmul.  All matmuls run in bf16 with fp32 PSUM.

v2 restructure (from trace analysis of the 448us baseline):
- scores PSUM is a [128, 784] 2-bank tile; ONE 784-wide Exp ACT per
  (head, q-half, k-tile) instead of four 512/32-wide ones (scalar engine
  fixed cost is ~352cyc/instr; this cuts scalar time ~40%).
- softmax 1/denom via DVE reciprocal_approx_fast instead of scalar
  Ln+Exp: removes 12 ACT_TABLE_LOAD bounces (~32us of scalar).
- V projection bias folded into the host-side output bias (rows of a
  normalized softmax sum to 1), removing per-head Identity ACTs.
- next pair's QKV/F matmuls are dribbled into the attention k-loop every
  few k-tiles so the PE never idles (HAM clock-gate stays warm).
- aug assembly copies moved to DMA (q rows, one-hot rows) or merged
  (both heads per F copy, V copy via strided dest) to unload the DVE.
"""

import os
import sys

import numpy as np

sys.path.insert(0, "/opt/trn_rl_repo")

B, D, H, W, C = 4, 8, 14, 14, 768
NH, HD = 12, 64
N = D * H * W  # 1568
HPC = 6  # heads per core
SCALE = HD ** -0.5
NKC = C // 128  # 6 k-chunks of input channels
NKT = (N + 127) // 128  # 13 token tiles (12x128 + 32)
FCH = [(0, 512), (512, 512), (1024, 512), (1536, 32)]  # free-dim chunks of N
QHW = 784  # q columns per attention half
HCH = [(0, 512), (512, 272)]  # free-dim chunks of a 784 half
KAUG = 100  # 64 qk channels + 8 + 14 + 14 bias channels

_CACHED = {}


def _build_nc():
    import concourse.bass as bass  # noqa: F401
    import concourse.mybir as mybir
    import concourse.tile as tile
    from concourse import bacc

    f32 = mybir.dt.float32
    bf16 = mybir.dt.bfloat16
    AF = mybir.ActivationFunctionType

    nc = bacc.Bacc(None, target_bir_lowering=False)

    # --- DRAM parameters (per-core shards; host pre-transposes/reorders) ---
    xT_d = nc.declare_dram_parameter("xT", [NKC, 128, N], bf16, isOutput=False)
    wqkv_d = nc.declare_dram_parameter("wqkv", [NKC, 128, 1152], bf16, isOutput=False)
    wproj_d = nc.declare_dram_parameter("wproj", [3, 128, 768], bf16, isOutput=False)
    oneh_d = nc.declare_dram_parameter("oneh", [36, N], bf16, isOutput=False)
    rdT_d = nc.declare_dram_parameter("rdT", [128, D * 40], bf16, isOutput=False)
    rhT_d = nc.declare_dram_parameter("rhT", [128, H * 46], bf16, isOutput=False)
    rwT_d = nc.declare_dram_parameter("rwT", [128, W * 46], bf16, isOutput=False)
    bqk_d = nc.declare_dram_parameter("bqk", [128, 6], f32, isOutput=False)
    out_d = nc.declare_dram_parameter("out", [NKC, 128, N], bf16, isOutput=True)

    with tile.TileContext(nc) as tc:
        with (
            tc.tile_pool(name="const", bufs=1) as cpool,
            tc.tile_pool(name="work", bufs=2) as wpool,
            tc.tile_pool(name="qkp", bufs=2, space="PSUM") as qkp,
            tc.tile_pool(name="avp", bufs=1, space="PSUM") as avp,
            tc.tile_pool(name="mmp", bufs=2, space="PSUM") as mmp,
        ):
            # ---- load constants ----
            xT = cpool.tile([128, NKC * N], bf16)
            wqkv = cpool.tile([128, NKC * 1152], bf16)
            wproj = cpool.tile([128, 3 * 768], bf16)
            oneh = cpool.tile([36, N], bf16)
            rdT = cpool.tile([128, D * 40], bf16)
            rhT = cpool.tile([128, H * 46], bf16)
            rwT = cpool.tile([128, W * 46], bf16)
            bqk = cpool.tile([128, 6], f32)
            # xT/wqkv land first (first matmuls need all of both); wproj last
            for kc in range(NKC):
                nc.sync.dma_start(xT[:, kc * N:(kc + 1) * N], xT_d[kc])
                nc.sync.dma_start(wqkv[:, kc * 1152:(kc + 1) * 1152], wqkv_d[kc])
            nc.sync.dma_start(bqk[:], bqk_d[:])
            nc.sync.dma_start(rdT[:], rdT_d[:])
            nc.sync.dma_start(rhT[:], rhT_d[:])
            nc.sync.dma_start(rwT[:], rwT_d[:])
            nc.sync.dma_start(oneh[:], oneh_d[:])
            for t3 in range(3):
                nc.sync.dma_start(wproj[:, t3 * 768:(t3 + 1) * 768], wproj_d[t3])

            # ---- HAM warm-up ----
            # The PE clock-gate defaults to 1.2 GHz and only opens to 2.4 GHz
            # after ~3.4us of sustained matmul activity.  The first real
            # matmul can't start until xT/wqkv land (~13us of DMA), and the
            # cold+oscillating clock then drags the whole prep phase.  Chew
            # through the DMA window with dependency-free dummy matmuls so
            # the real work starts at full clock.
            warm = cpool.tile([128, 512], bf16, name="warm")
            nc.vector.memset(warm[:], 0.0)
            for i in range(50):
                pw = mmp.tile([128, 512], f32, name="pw", tag="mm")
                nc.tensor.matmul(pw[:, 0:512], warm[:, 0:128], warm[:, 0:512])

            # ---- persistent SBUF state ----
            # V in natural [token, channel] layout, ones column per head
            vnat = cpool.tile([128, NKT, HPC * 65], bf16)
            av_all = [
                cpool.tile([128, N], bf16, name=f"av_all{i}", tag=f"av{i}")
                for i in range(3)
            ]

            # ---- deferred-work queue (dribbled into attention k-loops) ----
            pending = []

            def dribble():
                if pending:
                    pending.pop(0)()

            def emit_v(kt):
                kp = min(128, N - kt * 128)
                pv = mmp.tile([128, 512], f32, name="pv", tag="mm")
                for kc in range(NKC):
                    nc.tensor.matmul(
                        pv[0:kp, 0:384],
                        xT[:, kc * N + kt * 128: kc * N + kt * 128 + kp],
                        wqkv[:, kc * 1152 + 768: kc * 1152 + 1152],
                        start=(kc == 0), stop=(kc == NKC - 1),
                    )
                dst = vnat[0:kp, kt, :].rearrange("p (h c) -> p h c", h=HPC)
                src = pv[0:kp, 0:384].rearrange("p (h c) -> p h c", h=HPC)
                nc.vector.tensor_copy(dst[:, :, 0:64], src)

            # ---- QKV for one head pair (queued as closures) ----
            def emit_qkv(p, eager):
                augs = []
                for x in range(2):
                    q_t = wpool.tile([KAUG, N], bf16, name=f"qaug{x}", tag=f"qaug{x}")
                    k_t = wpool.tile([128, N], bf16, name=f"kaug{x}", tag=f"kaug{x}")
                    augs.append((q_t, k_t))
                qpair = wpool.tile([128, N], bf16, name="qpair", tag="qpair")

                def chunk(qk, f0, fl):
                    def run():
                        col0 = qk * 384 + p * 128
                        bcol = qk * 3 + p
                        ps = mmp.tile([128, 512], f32, name="ps", tag="mm")
                        for kc in range(NKC):
                            nc.tensor.matmul(
                                ps[:, 0:fl],
                                wqkv[:, kc * 1152 + col0: kc * 1152 + col0 + 128],
                                xT[:, kc * N + f0: kc * N + f0 + fl],
                                start=(kc == 0), stop=(kc == NKC - 1),
                            )
                        # pair 0's bias-adds ride the then-idle scalar engine
                        # so the DVE can assemble the aug tiles sooner
                        if qk == 0:
                            if eager:
                                nc.scalar.add(
                                    qpair[:, f0:f0 + fl], ps[:, 0:fl],
                                    bqk[:, bcol:bcol + 1],
                                )
                            else:
                                nc.vector.tensor_scalar_add(
                                    qpair[:, f0:f0 + fl], ps[:, 0:fl],
                                    bqk[:, bcol:bcol + 1],
                                )
                        else:
                            for x in range(2):
                                if eager:
                                    nc.scalar.add(
                                        augs[x][1][0:64, f0:f0 + fl],
                                        ps[x * 64:(x + 1) * 64, 0:fl],
                                        bqk[x * 64:(x + 1) * 64, bcol:bcol + 1],
                                    )
                                else:
                                    nc.vector.tensor_scalar_add(
                                        augs[x][1][0:64, f0:f0 + fl],
                                        ps[x * 64:(x + 1) * 64, 0:fl],
                                        bqk[x * 64:(x + 1) * 64, bcol:bcol + 1],
                                    )
                    return run

                def assemble():
                    for x in range(2):
                        nc.sync.dma_start(
                            augs[x][0][0:64, :], qpair[x * 64:(x + 1) * 64, :]
                        )
                        nc.sync.dma_start(augs[x][1][64:KAUG, :], oneh[:])

                jobs = [chunk(qk, f0, fl) for qk in range(2) for (f0, fl) in FCH]
                jobs.append(assemble)
                if eager:
                    for j in jobs:
                        j()
                else:
                    pending.extend(jobs)
                return augs, qpair

            # ---- F = q.R rel-pos tables for one pair (queued as closures) ----
            def emit_F(p, augs, qpair, eager):
                qpv = qpair.rearrange("p (d h w) -> p d h w", d=D, h=H, w=W)

                def rel_d():
                    fsD = wpool.tile([40, N], bf16, name="fsD", tag="fsD")
                    for qd in range(D):
                        pf = mmp.tile([128, 512], f32, name="pf", tag="mm")
                        nc.tensor.matmul(
                            pf[0:40, 0:H * W],
                            rdT[:, qd * 40:(qd + 1) * 40],
                            qpair[:, qd * H * W:(qd + 1) * H * W],
                        )
                        nc.vector.tensor_copy(
                            fsD[:, qd * H * W:(qd + 1) * H * W], pf[0:40, 0:H * W]
                        )
                    for x in range(2):
                        nc.sync.dma_start(augs[x][0][64:72, :], fsD[32 * x:32 * x + 8, :])

                def rel_h():
                    fsH = wpool.tile([46, N], bf16, name="fsH", tag="fsH")
                    fv = fsH.rearrange("p (d h w) -> p d h w", d=D, h=H, w=W)
                    for qh in range(H):
                        pf = mmp.tile([128, 512], f32, name="pf", tag="mm")
                        nc.tensor.matmul(
                            pf[0:46, 0:D * W], rhT[:, qh * 46:(qh + 1) * 46],
                            qpv[:, :, qh, :]
                        )
                        nc.vector.tensor_copy(fv[:, :, qh, :], pf[0:46, 0:D * W])
                    for x in range(2):
                        nc.sync.dma_start(augs[x][0][72:86, :], fsH[32 * x:32 * x + 14, :])

                def rel_w():
                    fsW = wpool.tile([46, N], bf16, name="fsW", tag="fsW")
                    fv = fsW.rearrange("p (d h w) -> p d h w", d=D, h=H, w=W)
                    for qw in range(W):
                        pf = mmp.tile([128, 512], f32, name="pf", tag="mm")
                        nc.tensor.matmul(
                            pf[0:46, 0:D * H], rwT[:, qw * 46:(qw + 1) * 46],
                            qpv[:, :, :, qw]
                        )
                        nc.vector.tensor_copy(fv[:, :, :, qw], pf[0:46, 0:D * H])
                    for x in range(2):
                        nc.sync.dma_start(augs[x][0][86:100, :], fsW[32 * x:32 * x + 14, :])

                jobs = [rel_d, rel_h, rel_w]
                if eager:
                    for j in jobs:
                        j()
                else:
                    pending.extend(jobs)

            # ---- attention for one head over one 784-wide q half ----
            # Attention is emitted as 12 "segments" (head x q-half), each a
            # 13-iteration k-loop, software-pipelined ACROSS segments: the AV
            # matmuls lag the QK matmuls by LAG k-tiles, and each segment's
            # first LAG QK+exp pairs are emitted before the previous
            # segment's tail AVs + drain.  Without this, the next segment's
            # first exp sits behind the previous segment's last AV (which
            # itself waits on an exp) in the PE's strict FIFO, stalling the
            # scalar engine ~1.5us at every one of the 24 boundaries.
            LAG = 2

            class Seg:
                def __init__(self, p, x, qh, avst, denb, jit_v):
                    self.p, self.x, self.qh = p, x, qh
                    self.avst, self.denb, self.jit_v = avst, denb, jit_v
                    self.h6 = 2 * p + x
                    self.q_t, self.k_t = augs_cur[x]
                    self.ets = {}
                    self.pav = None

                def qk_exp(self, kt):
                    kp = min(128, N - kt * 128)
                    if self.jit_v:
                        emit_v(kt)
                    elif (kt % 5 == 1 or (self.x == 1 and kt % 5 == 3)
                          or (self.p == 2 and self.x == 1 and self.qh == 1)):
                        dribble()
                    qk = qkp.tile([128, QHW], f32, name="qk", tag="qk")
                    for (f0, fl) in HCH:
                        nc.tensor.matmul(
                            qk[0:kp, f0:f0 + fl],
                            self.k_t[0:KAUG, kt * 128: kt * 128 + kp],
                            self.q_t[0:KAUG, self.qh * QHW + f0:
                                     self.qh * QHW + f0 + fl],
                        )
                    et = wpool.tile([128, QHW], bf16, name="et", tag="et", bufs=5)
                    nc.scalar.activation(et[0:kp, :], qk[0:kp, 0:QHW], AF.Exp)
                    self.ets[kt] = et

                def av(self, kt):
                    if self.pav is None:
                        self.pav = avp.tile([65, QHW], f32, name="pav", tag="av")
                    kp = min(128, N - kt * 128)
                    et = self.ets.pop(kt)
                    for (f0, fl) in HCH:
                        nc.tensor.matmul(
                            self.pav[:, f0:f0 + fl],
                            vnat[0:kp, kt, self.h6 * 65:(self.h6 + 1) * 65],
                            et[0:kp, f0:f0 + fl],
                            start=(kt == 0), stop=(kt == NKT - 1),
                        )

                def begin(self):
                    for kt in range(LAG):
                        self.qk_exp(kt)

                def body(self):
                    for kt in range(LAG, NKT):
                        self.qk_exp(kt)
                        self.av(kt - LAG)

                def finish(self):
                    for kt in range(NKT - LAG, NKT):
                        self.av(kt)
                    sl = slice(self.qh * QHW, (self.qh + 1) * QHW)
                    nc.vector.tensor_copy(self.avst[0:64, sl], self.pav[0:64, :])
                    nc.vector.tensor_copy(self.denb[0:1, sl], self.pav[64:65, :])
                    dribble()
                    emit_norm(self.p, self.x, self.qh, self.avst, self.denb)

            # ---- normalize one q-half of one head into av_all ----
            def emit_norm(p, x, qh, avst, denb):
                sl = slice(qh * QHW, (qh + 1) * QHW)
                rcp = wpool.tile([1, QHW], f32, name="rcp", tag="rcp")
                rbc = wpool.tile([64, QHW], f32, name="rbc", tag="rbc")
                nc.vector.reciprocal_approx_fast(rcp[0:1, :], denb[0:1, sl])
                nc.gpsimd.partition_broadcast(rbc[0:64, :], rcp[0:1, :])
                nc.vector.tensor_mul(
                    av_all[p][x * 64:(x + 1) * 64, sl], avst[0:64, sl], rbc[0:64, :]
                )

            # ---- projection helpers ----
            ots = [
                cpool.tile([128, N], bf16, name=f"ot{mo}", tag=f"out{mo}")
                for mo in range(NKC)
            ]

            def proj_chunk(mo, f0, fl, alt=0):
                def run():
                    # alternate PSUM pool and copy engine for a deeper
                    # tail pipeline (avp's banks are free by proj time)
                    pool = mmp if alt % 2 == 0 else avp
                    tag = "mm" if alt % 2 == 0 else "av"
                    ps = pool.tile([128, 512], f32, name="ps", tag=tag)
                    for t3 in range(3):
                        nc.tensor.matmul(
                            ps[:, 0:fl],
                            wproj[:, t3 * 768 + mo * 128: t3 * 768 + mo * 128 + 128],
                            av_all[t3][:, f0:f0 + fl],
                            start=(t3 == 0), stop=(t3 == 2),
                        )
                    if alt % 2 == 0:
                        nc.vector.tensor_copy(ots[mo][:, f0:f0 + fl], ps[:, 0:fl])
                    else:
                        nc.scalar.copy(ots[mo][:, f0:f0 + fl], ps[:, 0:fl])
                return run

            # ---- pipeline driver ----
            augs_cur, qpair_cur = emit_qkv(0, eager=True)
            emit_F(0, augs_cur, qpair_cur, eager=True)
            nc.vector.memset(vnat[:], 1.0)
            prev_seg = None
            for p in range(3):
                avsts = [
                    wpool.tile([64, N], f32, name=f"avst{x}", tag=f"avst{x}", bufs=1)
                    for x in range(2)
                ]
                denbs = [
                    wpool.tile([1, N], f32, name=f"denb{x}", tag=f"denb{x}", bufs=1)
                    for x in range(2)
                ]
                if p > 0:
                    augs_cur, qpair_cur = augs_nxt, qpair_nxt
                if p < 2:
                    augs_nxt, qpair_nxt = emit_qkv(p + 1, eager=False)
                    emit_F(p + 1, augs_nxt, qpair_nxt, eager=False)
                for x in range(2):
                    for qh in range(2):
                        seg = Seg(p, x, qh, avsts[x], denbs[x],
                                  jit_v=(p == 0 and x == 0 and qh == 0))
                        seg.begin()
                        if prev_seg is not None:
                            prev_seg.finish()
                        if p == 2 and x == 1 and qh == 1:
                            # proj cols 0:784 only need the qh=0 halves;
                            # pushed after (2,1,0)'s norm is emitted.
                            # alt=0 only: avp's slot is still held by pav.
                            pending.extend(
                                proj_chunk(mo, f0, fl, alt=0)
                                for mo in range(NKC)
                                for (f0, fl) in ((0, 512), (512, 272))
                            )
                        seg.body()
                        prev_seg = seg
                # pair boundary: the next pair's aug tiles must be fully
                # assembled (dribbled closures run) before its first QK
                if p < 2:
                    while pending:
                        dribble()
            prev_seg.finish()

            # ---- rest of the partial projection: outT[768, N] ----
            while pending:
                dribble()
            alt = 0
            for mo in range(NKC):
                for (f0, fl) in ((784, 240), (1024, 512), (1536, 32)):
                    proj_chunk(mo, f0, fl, alt=alt)()
                    alt += 1
                nc.sync.dma_start(out_d[mo], ots[mo][:])

    nc.compile()
    return nc


def _prep_inputs(x, qkv_w, qkv_b, proj_w, proj_b, rel_pos_d, rel_pos_h, rel_pos_w):
    """Host-side shard prep: returns in_maps list for 8 cores."""
    import ml_dtypes
    bf = ml_dtypes.bfloat16
    x = np.ascontiguousarray(x, np.float32)
    qkv_w = np.asarray(qkv_w, np.float32)
    qkv_b = np.asarray(qkv_b, np.float32)
    proj_w = np.asarray(proj_w, np.float32)

    # one-hot k-position rows [36, N]
    j = np.arange(N)
    kd, kh, kw = j // (H * W), (j // W) % H, j % W
    oneh = np.zeros((36, N), np.float32)
    oneh[kd, j] = 1.0
    oneh[8 + kh, j] = 1.0
    oneh[22 + kw, j] = 1.0
    oneh = oneh.astype(bf)

    # rel tables, transposed and un-scaled (q is pre-scaled by SCALE).
    # Block-diagonal over the head pair: head A channels in rows 0:64 feed
    # psum rows 0:n, head B channels in rows 64:128 feed psum rows 32:32+n.
    def rtab(table, n, span):
        t = np.asarray(table, np.float32) / SCALE  # [2n-1, 64]
        qq, kk = np.meshgrid(np.arange(n), np.arange(n), indexing="ij")
        base = t[(qq - kk + n - 1).reshape(-1)].T.reshape(64, n, n)  # [c, q, k]
        out = np.zeros((128, n, span), np.float32)
        out[0:64, :, 0:n] = base
        out[64:128, :, 32:32 + n] = base
        return np.ascontiguousarray(out.reshape(128, n * span)).astype(bf)

    rdT = rtab(rel_pos_d, D, 40)
    rhT = rtab(rel_pos_h, H, 46)
    rwT = rtab(rel_pos_w, W, 46)

    in_maps = []
    for core in range(8):
        b, g = divmod(core, 2)
        heads = list(range(g * HPC, (g + 1) * HPC))
        # W columns: [q(6x64) | k(6x64) | v(6x64)] for this head group; q scaled
        cols_q = [0 * C + h * HD + c for h in heads for c in range(HD)]
        cols_k = [1 * C + h * HD + c for h in heads for c in range(HD)]
        cols_v = [2 * C + h * HD + c for h in heads for c in range(HD)]
        wq = qkv_w[:, cols_q] * SCALE
        wk = qkv_w[:, cols_k]
        wv = qkv_w[:, cols_v]
        wc = np.concatenate([wq, wk, wv], axis=1)  # [768, 1152]
        wqkv = np.ascontiguousarray(wc.reshape(NKC, 128, 1152)).astype(bf)

        bq = qkv_b[cols_q] * SCALE
        bk = qkv_b[cols_k]
        bqk = np.zeros((128, 6), np.float32)
        for p in range(3):
            bqk[:, p] = bq[p * 128:(p + 1) * 128]
            bqk[:, 3 + p] = bk[p * 128:(p + 1) * 128]

        rows = [h * HD + c for h in heads for c in range(HD)]
        wp = np.ascontiguousarray(proj_w[rows].reshape(3, 128, 768)).astype(bf)

        xT = np.ascontiguousarray(
            x[b].reshape(N, C).T.reshape(NKC, 128, N)
        ).astype(bf)
        in_maps.append({
            "xT": xT, "wqkv": wqkv, "wproj": wp, "oneh": oneh,
            "rdT": rdT, "rhT": rhT, "rwT": rwT, "bqk": bqk,
        })
    return in_maps


def _install_ntff_hook_shim():
    """The image's antenv package lacks axon_hooks; recreate it so
    run_bass_kernel_spmd(trace=True) can reach the libaxon NTFF profiler."""
    import types

    if "antenv.axon_hooks" in sys.modules:
        return
    import antenv
    mod = types.ModuleType("antenv.axon_hooks")
    _hook = [None]
    mod.set_axon_ntff_profile_hook = lambda h: _hook.__setitem__(0, h)
    mod.get_axon_ntff_profile_hook = lambda: _hook[0]
    antenv.axon_hooks = mod
    sys.modules["antenv.axon_hooks"] = mod
    try:
        from trn_agent_boot.trn_boot import _ntff_profile_via_ctypes

        mod.set_axon_ntff_profile_hook(
            _ntff_profile_via_ctypes("/opt/axon/libaxon_pjrt.so")
        )
    except Exception as e:  # degrade to no tracing
        print(f"ntff hook shim failed: {e}", file=sys.stderr)


def _patch_ldw_opt():
    """Enable walrus's LDWEIGHTS dedup (consecutive matmuls reusing the same
    stationary operand skip the reload). bass_utils hardcodes it off.
    Disabled: walrus codegen fails with it on for this BIR."""
    if not bool(int(os.environ.get("KERNEL_LDW_OPT", "0"))):
        return
    import concourse.bass_utils as bu

    if getattr(bu, "_ldw_patched", False):
        return
    orig = bu.run_command

    def run_command(cmd, *a, **kw):
        if isinstance(cmd, list):
            cmd = [
                "--enable-ldw-opt=true" if c == "--enable-ldw-opt=false" else c
                for c in cmd
            ]
        return orig(cmd, *a, **kw)

    bu.run_command = run_command
    bu._ldw_patched = True


LAST_EXEC_NS = None


def kernel(x, qkv_w, qkv_b, proj_w, proj_b, rel_pos_d, rel_pos_h, rel_pos_w):
    global LAST_EXEC_NS
    if "nc" not in _CACHED:
        _CACHED["nc"] = _build_nc()
    nc = _CACHED["nc"]
    in_maps = _prep_inputs(
        x, qkv_w, qkv_b, proj_w, proj_b, rel_pos_d, rel_pos_h, rel_pos_w
    )
    _patch_ldw_opt()
    from concourse.bass_utils import run_bass_kernel_spmd

    trace = bool(int(os.environ.get("KERNEL_TRACE", "0")))
    if trace:
        _install_ntff_hook_shim()
    res = run_bass_kernel_spmd(nc, in_maps, core_ids=list(range(8)), trace=trace)
    LAST_EXEC_NS = res.exec_time_ns
    # V-projection bias: softmax rows sum to 1, so attn @ (v + bv) =
    # attn @ v + bv; bv then passes through proj as a constant vector.
    proj_b = np.asarray(proj_b, np.float32)
    qkv_b = np.asarray(qkv_b, np.float32)
    proj_w = np.asarray(proj_w, np.float32)
    bias_full = proj_b + qkv_b[2 * C:] @ proj_w
    outs = []
    for b in range(B):
        t0 = res.results[2 * b]["out"].reshape(C, N).astype(np.float32)
        t1 = res.results[2 * b + 1]["out"].reshape(C, N).astype(np.float32)
        outs.append((t0 + t1).T + bias_full)
    return np.stack(outs).reshape(B, D, H, W, C).astype(np.float32)


# revision 60
# speedup vs baseline: 1.2808x; 1.0071x over previous
"""Distributed Bass kernel: 3D windowed attention with decomposed rel-pos bias.

Sharding: 8 cores = 4 batches x 2 head-groups (6 heads each).
Per-core layout is fully transposed ([channel, token]); the rel-pos bias is
folded into the scores matmul as 36 extra contraction channels (one-hot
k-position rows in the stationary operand, F = q.R tables in the moving
operand).  Softmax runs max-free; the denominator comes free as a ones-row
appended to V in the AV matmul.  All matmuls run in bf16 with fp32 PSUM.

v2 restructure (from trace analysis of the 448us baseline):
- scores PSUM is a [128, 784] 2-bank tile; ONE 784-wide Exp ACT per
  (head, q-half, k-tile) instead of four 512/32-wide ones (scalar engine
  fixed cost is ~352cyc/instr; this cuts scalar time ~40%).
- softmax 1/denom via DVE reciprocal_approx_fast instead of scalar
  Ln+Exp: removes 12 ACT_TABLE_LOAD bounces (~32us of scalar).
- V projection bias folded into the host-side output bias (rows of a
  normalized softmax sum to 1), removing per-head Identity ACTs.
- next pair's QKV/F matmuls are dribbled into the attention k-loop every
  few k-tiles so the PE never idles (HAM clock-gate stays warm).
- aug assembly copies moved to DMA (q rows, one-hot rows) or merged
  (both heads per F copy, V copy via strided dest) to unload the DVE.
"""

import os
import sys

import numpy as np

sys.path.insert(0, "/opt/trn_rl_repo")

B, D, H, W, C = 4, 8, 14, 14, 768
NH, HD = 12, 64
N = D * H * W  # 1568
HPC = 6  # heads per core
SCALE = HD ** -0.5
NKC = C // 128  # 6 k-chunks of input channels
NKT = (N + 127) // 128  # 13 token tiles (12x128 + 32)
FCH = [(0, 512), (512, 512), (1024, 512), (1536, 32)]  # free-dim chunks of N
QHW = 784  # q columns per attention half
HCH = [(0, 512), (512, 272)]  # free-dim chunks of a 784 half
KAUG = 100  # 64 qk channels + 8 + 14 + 14 bias channels

_CACHED = {}


def _build_nc():
    import concourse.bass as bass  # noqa: F401
    import concourse.mybir as mybir
    import concourse.tile as tile
    from concourse import bacc

    f32 = mybir.dt.float32
    bf16 = mybir.dt.bfloat16
    AF = mybir.ActivationFunctionType

    nc = bacc.Bacc(None, target_bir_lowering=False)

    # --- DRAM parameters (per-core shards; host pre-transposes/reorders) ---
    xT_d = nc.declare_dram_parameter("xT", [NKC, 128, N], bf16, isOutput=False)
    wqkv_d = nc.declare_dram_parameter("wqkv", [NKC, 128, 1152], bf16, isOutput=False)
    wproj_d = nc.declare_dram_parameter("wproj", [3, 128, 768], bf16, isOutput=False)
    oneh_d = nc.declare_dram_parameter("oneh", [36, N], bf16, isOutput=False)
    rdT_d = nc.declare_dram_parameter("rdT", [128, D * 40], bf16, isOutput=False)
    rhT_d = nc.declare_dram_parameter("rhT", [128, H * 46], bf16, isOutput=False)
    rwT_d = nc.declare_dram_parameter("rwT", [128, W * 46], bf16, isOutput=False)
    bqk_d = nc.declare_dram_parameter("bqk", [128, 6], f32, isOutput=False)
    out_d = nc.declare_dram_parameter("out", [NKC, 128, N], bf16, isOutput=True)

    with tile.TileContext(nc) as tc:
        with (
            tc.tile_pool(name="const", bufs=1) as cpool,
            tc.tile_pool(name="work", bufs=2) as wpool,
            tc.tile_pool(name="qkp", bufs=2, space="PSUM") as qkp,
            tc.tile_pool(name="avp", bufs=1, space="PSUM") as avp,
            tc.tile_pool(name="mmp", bufs=2, space="PSUM") as mmp,
        ):
            # ---- load constants ----
            xT = cpool.tile([128, NKC * N], bf16)
            wqkv = cpool.tile([128, NKC * 1152], bf16)
            wproj = cpool.tile([128, 3 * 768], bf16)
            oneh = cpool.tile([36, N], bf16)
            rdT = cpool.tile([128, D * 40], bf16)
            rhT = cpool.tile([128, H * 46], bf16)
            rwT = cpool.tile([128, W * 46], bf16)
            bqk = cpool.tile([128, 6], f32)
            # xT/wqkv land first (first matmuls need all of both); wproj last
            for kc in range(NKC):
                nc.sync.dma_start(xT[:, kc * N:(kc + 1) * N], xT_d[kc])
                nc.sync.dma_start(wqkv[:, kc * 1152:(kc + 1) * 1152], wqkv_d[kc])
            nc.sync.dma_start(bqk[:], bqk_d[:])
            nc.sync.dma_start(rdT[:], rdT_d[:])
            nc.sync.dma_start(rhT[:], rhT_d[:])
            nc.sync.dma_start(rwT[:], rwT_d[:])
            nc.sync.dma_start(oneh[:], oneh_d[:])
            for t3 in range(3):
                nc.sync.dma_start(wproj[:, t3 * 768:(t3 + 1) * 768], wproj_d[t3])

            # ---- HAM warm-up ----
            # The PE clock-gate defaults to 1.2 GHz and only opens to 2.4 GHz
            # after ~3.4us of sustained matmul activity.  The first real
            # matmul can't start until xT/wqkv land (~13us of DMA), and the
            # cold+oscillating clock then drags the whole prep phase.  Chew
            # through the DMA window with dependency-free dummy matmuls so
            # the real work starts at full clock.
            warm = cpool.tile([128, 512], bf16, name="warm")
            nc.vector.memset(warm[:], 0.0)
            for i in range(50):
                pw = mmp.tile([128, 512], f32, name="pw", tag="mm")
                nc.tensor.matmul(pw[:, 0:512], warm[:, 0:128], warm[:, 0:512])

            # ---- persistent SBUF state ----
            # V in natural [token, channel] layout, ones column per head
            vnat = cpool.tile([128, NKT, HPC * 65], bf16)
            av_all = [
                cpool.tile([128, N], bf16, name=f"av_all{i}", tag=f"av{i}")
                for i in range(3)
            ]

            # ---- deferred-work queue (dribbled into attention k-loops) ----
            pending = []

            def dribble():
                if pending:
                    pending.pop(0)()

            def emit_v(kt):
                kp = min(128, N - kt * 128)
                pv = mmp.tile([128, 512], f32, name="pv", tag="mm")
                for kc in range(NKC):
                    nc.tensor.matmul(
                        pv[0:kp, 0:384],
                        xT[:, kc * N + kt * 128: kc * N + kt * 128 + kp],
                        wqkv[:, kc * 1152 + 768: kc * 1152 + 1152],
                        start=(kc == 0), stop=(kc == NKC - 1),
                    )
                dst = vnat[0:kp, kt, :].rearrange("p (h c) -> p h c", h=HPC)
                src = pv[0:kp, 0:384].rearrange("p (h c) -> p h c", h=HPC)
                nc.vector.tensor_copy(dst[:, :, 0:64], src)

            # ---- QKV for one head pair (queued as closures) ----
            def emit_qkv(p, eager):
                augs = []
                for x in range(2):
                    q_t = wpool.tile([KAUG, N], bf16, name=f"qaug{x}", tag=f"qaug{x}")
                    k_t = wpool.tile([128, N], bf16, name=f"kaug{x}", tag=f"kaug{x}")
                    augs.append((q_t, k_t))
                qpair = wpool.tile([128, N], bf16, name="qpair", tag="qpair")

                def chunk(qk, f0, fl):
                    def run():
                        col0 = qk * 384 + p * 128
                        bcol = qk * 3 + p
                        ps = mmp.tile([128, 512], f32, name="ps", tag="mm")
                        for kc in range(NKC):
                            nc.tensor.matmul(
                                ps[:, 0:fl],
                                wqkv[:, kc * 1152 + col0: kc * 1152 + col0 + 128],
                                xT[:, kc * N + f0: kc * N + f0 + fl],
                                start=(kc == 0), stop=(kc == NKC - 1),
                            )
                        # pair 0's bias-adds ride the then-idle scalar engine
                        # so the DVE can assemble the aug tiles sooner
                        if qk == 0:
                            if eager:
                                nc.scalar.add(
                                    qpair[:, f0:f0 + fl], ps[:, 0:fl],
                                    bqk[:, bcol:bcol + 1],
                                )
                            else:
                                nc.vector.tensor_scalar_add(
                                    qpair[:, f0:f0 + fl], ps[:, 0:fl],
                                    bqk[:, bcol:bcol + 1],
                                )
                        else:
                            for x in range(2):
                                if eager:
                                    nc.scalar.add(
                                        augs[x][1][0:64, f0:f0 + fl],
                                        ps[x * 64:(x + 1) * 64, 0:fl],
                                        bqk[x * 64:(x + 1) * 64, bcol:bcol + 1],
                                    )
                                else:
                                    nc.vector.tensor_scalar_add(
                                        augs[x][1][0:64, f0:f0 + fl],
                                        ps[x * 64:(x + 1) * 64, 0:fl],
                                        bqk[x * 64:(x + 1) * 64, bcol:bcol + 1],
                                    )
                    return run

                def assemble():
                    for x in range(2):
                        nc.sync.dma_start(
                            augs[x][0][0:64, :], qpair[x * 64:(x + 1) * 64, :]
                        )
                        nc.sync.dma_start(augs[x][1][64:KAUG, :], oneh[:])

                jobs = [chunk(qk, f0, fl) for qk in range(2) for (f0, fl) in FCH]
                jobs.append(assemble)
                if eager:
                    for j in jobs:
                        j()
                else:
                    pending.extend(jobs)
                return augs, qpair

            # ---- F = q.R rel-pos tables for one pair (queued as closures) ----
            def emit_F(p, augs, qpair, eager):
                qpv = qpair.rearrange("p (d h w) -> p d h w", d=D, h=H, w=W)

                def rel_d():
                    fsD = wpool.tile([40, N], bf16, name="fsD", tag="fsD")
                    for qd in range(D):
                        pf = mmp.tile([128, 512], f32, name="pf", tag="mm")
                        nc.tensor.matmul(
                            pf[0:40, 0:H * W],
                            rdT[:, qd * 40:(qd + 1) * 40],
                            qpair[:, qd * H * W:(qd + 1) * H * W],
                        )
                        nc.vector.tensor_copy(
                            fsD[:, qd * H * W:(qd + 1) * H * W], pf[0:40, 0:H * W]
                        )
                    for x in range(2):
                        nc.sync.dma_start(augs[x][0][64:72, :], fsD[32 * x:32 * x + 8, :])

                def rel_h():
                    fsH = wpool.tile([46, N], bf16, name="fsH", tag="fsH")
                    fv = fsH.rearrange("p (d h w) -> p d h w", d=D, h=H, w=W)
                    for qh in range(H):
                        pf = mmp.tile([128, 512], f32, name="pf", tag="mm")
                        nc.tensor.matmul(
                            pf[0:46, 0:D * W], rhT[:, qh * 46:(qh + 1) * 46],
                            qpv[:, :, qh, :]
                        )
                        nc.vector.tensor_copy(fv[:, :, qh, :], pf[0:46, 0:D * W])
                    for x in range(2):
                        nc.sync.dma_start(augs[x][0][72:86, :], fsH[32 * x:32 * x + 14, :])

                def rel_w():
                    fsW = wpool.tile([46, N], bf16, name="fsW", tag="fsW")
                    fv = fsW.rearrange("p (d h w) -> p d h w", d=D, h=H, w=W)
                    for qw in range(W):
                        pf = mmp.tile([128, 512], f32, name="pf", tag="mm")
                        nc.tensor.matmul(
                            pf[0:46, 0:D * H], rwT[:, qw * 46:(qw + 1) * 46],
                            qpv[:, :, :, qw]
                        )
                        nc.vector.tensor_copy(fv[:, :, :, qw], pf[0:46, 0:D * H])
                    for x in range(2):
                        nc.sync.dma_start(augs[x][0][86:100, :], fsW[32 * x:32 * x + 14, :])

                jobs = [rel_d, rel_h, rel_w]
                if eager:
                    for j in jobs:
                        j()
                else:
                    pending.extend(jobs)

            # ---- attention for one head over one 784-wide q half ----
            # Attention is emitted as 12 "segments" (head x q-half), each a
            # 13-iteration k-loop, software-pipelined ACROSS segments: the AV
            # matmuls lag the QK matmuls by LAG k-tiles, and each segment's
            # first LAG QK+exp pairs are emitted before the previous
            # segment's tail AVs + drain.  Without this, the next segment's
            # first exp sits behind the previous segment's last AV (which
            # itself waits on an exp) in the PE's strict FIFO, stalling the
            # scalar engine ~1.5us at every one of the 24 boundaries.
            LAG = 2

            class Seg:
                def __init__(self, p, x, qh, avst, denb, jit_v):
                    self.p, self.x, self.qh = p, x, qh
                    self.avst, self.denb, self.jit_v = avst, denb, jit_v
                    self.h6 = 2 * p + x
                    self.q_t, self.k_t = augs_cur[x]
                    self.ets = {}
                    self.pav = None

                def qk_exp(self, kt):
                    kp = min(128, N - kt * 128)
                    if self.jit_v:
                        if kt + 4 < NKT:
                            emit_v(kt + 4)
                    elif (kt % 5 == 1 or (self.x == 1 and kt % 5 == 3)
                          or (self.p == 2 and self.x == 1 and self.qh == 1)):
                        dribble()
                    qk = qkp.tile([128, QHW], f32, name="qk", tag="qk")
                    for (f0, fl) in HCH:
                        nc.tensor.matmul(
                            qk[0:kp, f0:f0 + fl],
                            self.k_t[0:KAUG, kt * 128: kt * 128 + kp],
                            self.q_t[0:KAUG, self.qh * QHW + f0:
                                     self.qh * QHW + f0 + fl],
                        )
                    et = wpool.tile([128, QHW], bf16, name="et", tag="et", bufs=5)
                    nc.scalar.activation(et[0:kp, :], qk[0:kp, 0:QHW], AF.Exp)
                    self.ets[kt] = et

                def av(self, kt):
                    if self.pav is None:
                        self.pav = avp.tile([65, QHW], f32, name="pav", tag="av")
                    kp = min(128, N - kt * 128)
                    et = self.ets.pop(kt)
                    for (f0, fl) in HCH:
                        nc.tensor.matmul(
                            self.pav[:, f0:f0 + fl],
                            vnat[0:kp, kt, self.h6 * 65:(self.h6 + 1) * 65],
                            et[0:kp, f0:f0 + fl],
                            start=(kt == 0), stop=(kt == NKT - 1),
                        )

                def begin(self):
                    if self.jit_v:
                        # V groups 0..3 fill the PE while the pair-0 aug
                        # tiles finish assembling (keeps the HAM gate open);
                        # the rest are emitted one per k-iteration.
                        for kt in range(4):
                            emit_v(kt)
                    for kt in range(LAG):
                        self.qk_exp(kt)

                def body(self):
                    for kt in range(LAG, NKT):
                        self.qk_exp(kt)
                        self.av(kt - LAG)

                def finish(self):
                    for kt in range(NKT - LAG, NKT):
                        self.av(kt)
                    sl = slice(self.qh * QHW, (self.qh + 1) * QHW)
                    nc.vector.tensor_copy(self.avst[0:64, sl], self.pav[0:64, :])
                    nc.vector.tensor_copy(self.denb[0:1, sl], self.pav[64:65, :])
                    dribble()
                    emit_norm(self.p, self.x, self.qh, self.avst, self.denb)

            # ---- normalize one q-half of one head into av_all ----
            def emit_norm(p, x, qh, avst, denb):
                sl = slice(qh * QHW, (qh + 1) * QHW)
                rcp = wpool.tile([1, QHW], f32, name="rcp", tag="rcp")
                rbc = wpool.tile([64, QHW], f32, name="rbc", tag="rbc")
                nc.vector.reciprocal_approx_fast(rcp[0:1, :], denb[0:1, sl])
                nc.gpsimd.partition_broadcast(rbc[0:64, :], rcp[0:1, :])
                nc.vector.tensor_mul(
                    av_all[p][x * 64:(x + 1) * 64, sl], avst[0:64, sl], rbc[0:64, :]
                )

            # ---- projection helpers ----
            ots = [
                cpool.tile([128, N], bf16, name=f"ot{mo}", tag=f"out{mo}")
                for mo in range(NKC)
            ]

            def proj_chunk(mo, f0, fl, alt=0):
                def run():
                    # alternate PSUM pool and copy engine for a deeper
                    # tail pipeline (avp's banks are free by proj time)
                    pool = mmp if alt % 2 == 0 else avp
                    tag = "mm" if alt % 2 == 0 else "av"
                    ps = pool.tile([128, 512], f32, name="ps", tag=tag)
                    for t3 in range(3):
                        nc.tensor.matmul(
                            ps[:, 0:fl],
                            wproj[:, t3 * 768 + mo * 128: t3 * 768 + mo * 128 + 128],
                            av_all[t3][:, f0:f0 + fl],
                            start=(t3 == 0), stop=(t3 == 2),
                        )
                    if alt % 2 == 0:
                        nc.vector.tensor_copy(ots[mo][:, f0:f0 + fl], ps[:, 0:fl])
                    else:
                        nc.scalar.copy(ots[mo][:, f0:f0 + fl], ps[:, 0:fl])
                return run

            # ---- pipeline driver ----
            augs_cur, qpair_cur = emit_qkv(0, eager=True)
            emit_F(0, augs_cur, qpair_cur, eager=True)
            nc.vector.memset(vnat[:], 1.0)
            prev_seg = None
            for p in range(3):
                avsts = [
                    wpool.tile([64, N], f32, name=f"avst{x}", tag=f"avst{x}", bufs=1)
                    for x in range(2)
                ]
                denbs = [
                    wpool.tile([1, N], f32, name=f"denb{x}", tag=f"denb{x}", bufs=1)
                    for x in range(2)
                ]
                if p > 0:
                    augs_cur, qpair_cur = augs_nxt, qpair_nxt
                if p < 2:
                    augs_nxt, qpair_nxt = emit_qkv(p + 1, eager=False)
                    emit_F(p + 1, augs_nxt, qpair_nxt, eager=False)
                for x in range(2):
                    for qh in range(2):
                        seg = Seg(p, x, qh, avsts[x], denbs[x],
                                  jit_v=(p == 0 and x == 0 and qh == 0))
                        seg.begin()
                        if prev_seg is not None:
                            prev_seg.finish()
                        if p == 2 and x == 1 and qh == 1:
                            # proj cols 0:784 only need the qh=0 halves;
                            # pushed after (2,1,0)'s norm is emitted.
                            # alt=0 only: avp's slot is still held by pav.
                            pending.extend(
                                proj_chunk(mo, f0, fl, alt=0)
                                for mo in range(NKC)
                                for (f0, fl) in ((0, 512), (512, 272))
                            )
                        seg.body()
                        prev_seg = seg
                # pair boundary: the next pair's aug tiles must be fully
                # assembled (dribbled closures run) before its first QK
                if p < 2:
                    while pending:
                        dribble()
            prev_seg.finish()

            # ---- rest of the partial projection: outT[768, N] ----
            while pending:
                dribble()
            alt = 0
            for mo in range(NKC):
                for (f0, fl) in ((784, 240), (1024, 512), (1536, 32)):
                    proj_chunk(mo, f0, fl, alt=alt)()
                    alt += 1
                nc.sync.dma_start(out_d[mo], ots[mo][:])

    nc.compile()
    return nc


def _prep_inputs(x, qkv_w, qkv_b, proj_w, proj_b, rel_pos_d, rel_pos_h, rel_pos_w):
    """Host-side shard prep: returns in_maps list for 8 cores."""
    import ml_dtypes
    bf = ml_dtypes.bfloat16
    x = np.ascontiguousarray(x, np.float32)
    qkv_w = np.asarray(qkv_w, np.float32)
    qkv_b = np.asarray(qkv_b, np.float32)
    proj_w = np.asarray(proj_w, np.float32)

    # one-hot k-position rows [36, N]
    j = np.arange(N)
    kd, kh, kw = j // (H * W), (j // W) % H, j % W
    oneh = np.zeros((36, N), np.float32)
    oneh[kd, j] = 1.0
    oneh[8 + kh, j] = 1.0
    oneh[22 + kw, j] = 1.0
    oneh = oneh.astype(bf)

    # rel tables, transposed and un-scaled (q is pre-scaled by SCALE).
    # Block-diagonal over the head pair: head A channels in rows 0:64 feed
    # psum rows 0:n, head B channels in rows 64:128 feed psum rows 32:32+n.
    def rtab(table, n, span):
        t = np.asarray(table, np.float32) / SCALE  # [2n-1, 64]
        qq, kk = np.meshgrid(np.arange(n), np.arange(n), indexing="ij")
        base = t[(qq - kk + n - 1).reshape(-1)].T.reshape(64, n, n)  # [c, q, k]
        out = np.zeros((128, n, span), np.float32)
        out[0:64, :, 0:n] = base
        out[64:128, :, 32:32 + n] = base
        return np.ascontiguousarray(out.reshape(128, n * span)).astype(bf)

    rdT = rtab(rel_pos_d, D, 40)
    rhT = rtab(rel_pos_h, H, 46)
    rwT = rtab(rel_pos_w, W, 46)

    in_maps = []
    for core in range(8):
        b, g = divmod(core, 2)
        heads = list(range(g * HPC, (g + 1) * HPC))
        # W columns: [q(6x64) | k(6x64) | v(6x64)] for this head group; q scaled
        cols_q = [0 * C + h * HD + c for h in heads for c in range(HD)]
        cols_k = [1 * C + h * HD + c for h in heads for c in range(HD)]
        cols_v = [2 * C + h * HD + c for h in heads for c in range(HD)]
        wq = qkv_w[:, cols_q] * SCALE
        wk = qkv_w[:, cols_k]
        wv = qkv_w[:, cols_v]
        wc = np.concatenate([wq, wk, wv], axis=1)  # [768, 1152]
        wqkv = np.ascontiguousarray(wc.reshape(NKC, 128, 1152)).astype(bf)

        bq = qkv_b[cols_q] * SCALE
        bk = qkv_b[cols_k]
        bqk = np.zeros((128, 6), np.float32)
        for p in range(3):
            bqk[:, p] = bq[p * 128:(p + 1) * 128]
            bqk[:, 3 + p] = bk[p * 128:(p + 1) * 128]

        rows = [h * HD + c for h in heads for c in range(HD)]
        wp = np.ascontiguousarray(proj_w[rows].reshape(3, 128, 768)).astype(bf)

        xT = np.ascontiguousarray(
            x[b].reshape(N, C).T.reshape(NKC, 128, N)
        ).astype(bf)
        in_maps.append({
            "xT": xT, "wqkv": wqkv, "wproj": wp, "oneh": oneh,
            "rdT": rdT, "rhT": rhT, "rwT": rwT, "bqk": bqk,
        })
    return in_maps


def _install_ntff_hook_shim():
    """The image's antenv package lacks axon_hooks; recreate it so
    run_bass_kernel_spmd(trace=True) can reach the libaxon NTFF profiler."""
    import types

    if "antenv.axon_hooks" in sys.modules:
        return
    import antenv
    mod = types.ModuleType("antenv.axon_hooks")
    _hook = [None]
    mod.set_axon_ntff_profile_hook = lambda h: _hook.__setitem__(0, h)
    mod.get_axon_ntff_profile_hook = lambda: _hook[0]
    antenv.axon_hooks = mod
    sys.modules["antenv.axon_hooks"] = mod
    try:
        from trn_agent_boot.trn_boot import _ntff_profile_via_ctypes

        mod.set_axon_ntff_profile_hook(
            _ntff_profile_via_ctypes("/opt/axon/libaxon_pjrt.so")
        )
    except Exception as e:  # degrade to no tracing
        print(f"ntff hook shim failed: {e}", file=sys.stderr)


def _patch_ldw_opt():
    """Enable walrus's LDWEIGHTS dedup (consecutive matmuls reusing the same
    stationary operand skip the reload). bass_utils hardcodes it off.
    Disabled: walrus codegen fails with it on for this BIR."""
    if not bool(int(os.environ.get("KERNEL_LDW_OPT", "0"))):
        return
    import concourse.bass_utils as bu

    if getattr(bu, "_ldw_patched", False):
        return
    orig = bu.run_command

    def run_command(cmd, *a, **kw):
        if isinstance(cmd, list):
            cmd = [
                "--enable-ldw-opt=true" if c == "--enable-ldw-opt=false" else c
                for c in cmd
            ]
        return orig(cmd, *a, **kw)

    bu.run_command = run_command
    bu._ldw_patched = True


LAST_EXEC_NS = None


def kernel(x, qkv_w, qkv_b, proj_w, proj_b, rel_pos_d, rel_pos_h, rel_pos_w):
    global LAST_EXEC_NS
    if "nc" not in _CACHED:
        _CACHED["nc"] = _build_nc()
    nc = _CACHED["nc"]
    in_maps = _prep_inputs(
        x, qkv_w, qkv_b, proj_w, proj_b, rel_pos_d, rel_pos_h, rel_pos_w
    )
    _patch_ldw_opt()
    from concourse.bass_utils import run_bass_kernel_spmd

    trace = bool(int(os.environ.get("KERNEL_TRACE", "0")))
    if trace:
        _install_ntff_hook_shim()
    res = run_bass_kernel_spmd(nc, in_maps, core_ids=list(range(8)), trace=trace)
    LAST_EXEC_NS = res.exec_time_ns
    # V-projection bias: softmax rows sum to 1, so attn @ (v + bv) =
    # attn @ v + bv; bv then passes through proj as a constant vector.
    proj_b = np.asarray(proj_b, np.float32)
    qkv_b = np.asarray(qkv_b, np.float32)
    proj_w = np.asarray(proj_w, np.float32)
    bias_full = proj_b + qkv_b[2 * C:] @ proj_w
    outs = []
    for b in range(B):
        t0 = res.results[2 * b]["out"].reshape(C, N).astype(np.float32)
        t1 = res.results[2 * b + 1]["out"].reshape(C, N).astype(np.float32)
        outs.append((t0 + t1).T + bias_full)
    return np.stack(outs).reshape(B, D, H, W, C).astype(np.float32)
